# revision 20
# baseline (speedup 1.0000x reference)
"""Trainium2 Bass kernel for nn_AGSISpaBlock (pre-norm MHA + GELU FFN block).

Sharding: 8 cores; core c handles batch b = c//2 and query-half qh = c%2.
Each core receives its batch's tokens PERMUTED so its 2048 local query rows
come first (attention is permutation-invariant over keys, so one SPMD graph
serves all cores). No collectives needed.

Dataflow on each core (all matmuls bf16 with fp32 PSUM accumulation):
  head:  LN1 (token-major, bn_stats, per-region batched sqrt) -> xn^T via PE
         transposes; Q^T/K^T projections; V token-major with importance
         folded in as exp(importance) row scaling (V'' = eimp * [V | 1]).
         LN1 and projections interleaved per 512-token region.
  attn:  keys-major scores S^T[k, q] via head-pair row-packed matmuls
         (heads 2p/2p+1 on PE rows 0-63/64-127 concurrently); exp(S/8) on
         ScalarE straight out of PSUM in 3/2-bank alternating groups;
         ctx^T[hd+1, q] accumulated over key chunks (ones column of V''
         carries the softmax denominator L).  qb-outer / pair-inner loop so
         each 512-query block finishes attention early.
  tail:  per qb: transpose ctx to token-major (L becomes a per-partition
         column) -> 1/L normalize -> transpose back -> Wo -> residual ->
         LN2 -> FFN (exact-erf Gelu) -> final residual -> DMA out.  All
         tail PSUM work goes through one reserved bank so it overlaps the
         attention of later query blocks.
"""

import sys

if "/opt/trn_rl_repo" not in sys.path:
    sys.path.insert(0, "/opt/trn_rl_repo")

import numpy as np
import ml_dtypes

import concourse.bass as bass
import concourse.tile as tile
from concourse import bacc, mybir
from concourse.bass_utils import run_bass_kernel_spmd

F32 = mybir.dt.float32
BF16 = mybir.dt.bfloat16
BF = ml_dtypes.bfloat16

B, N, D = 4, 4096, 256
H, HD = 4, 64
FF = 512
EPS = 1e-5
NQ = N // 2          # local queries per core
KC = N // 128        # key chunks (32)
QB = NQ // 512       # 512-wide query blocks (4)
QT = NQ // 128       # 128-wide query tiles (16)
NR = N // 512        # 512-token regions (8)

_compiled = None


def _build():
    nc = bacc.Bacc("TRN2", target_bir_lowering=False, debug=False, num_devices=8)

    tok = nc.declare_dram_parameter("tokens", [N, D], F32, isOutput=False)
    imp = nc.declare_dram_parameter("imp", [128, KC], F32, isOutput=False)
    wq = nc.declare_dram_parameter("wq", [D, D], BF16, isOutput=False)
    wk = nc.declare_dram_parameter("wk", [D, D], BF16, isOutput=False)
    wv = nc.declare_dram_parameter("wv", [D, D], BF16, isOutput=False)
    wo = nc.declare_dram_parameter("wo", [D, D], BF16, isOutput=False)
    w1 = nc.declare_dram_parameter("w1", [D, FF], BF16, isOutput=False)
    w2 = nc.declare_dram_parameter("w2", [FF, D], BF16, isOutput=False)
    bq = nc.declare_dram_parameter("bq", [128, 2], F32, isOutput=False)
    bk = nc.declare_dram_parameter("bk", [128, 2], F32, isOutput=False)
    bva = nc.declare_dram_parameter("bva", [1, H * 65], BF16, isOutput=False)
    bo = nc.declare_dram_parameter("bo", [128, 2], F32, isOutput=False)
    b1 = nc.declare_dram_parameter("b1", [128, 4], F32, isOutput=False)
    b2 = nc.declare_dram_parameter("b2", [128, 2], F32, isOutput=False)
    idb = nc.declare_dram_parameter("idb", [128, 128], BF16, isOutput=False)
    out = nc.declare_dram_parameter("out", [NQ, D], F32, isOutput=True)

    EXP = mybir.ActivationFunctionType.Exp
    GELU = mybir.ActivationFunctionType.Gelu
    SQRT = mybir.ActivationFunctionType.Sqrt
    SUB = mybir.AluOpType.subtract
    MUL = mybir.AluOpType.mult
    ADD = mybir.AluOpType.add

    with tile.TileContext(nc) as tc:
        with (
            tc.tile_pool(name="singles", bufs=1) as S,
            tc.tile_pool(name="work", bufs=4) as W4,
            tc.tile_pool(name="stats", bufs=4) as ST,
        ):
            # ---- persistent SBUF tensors (chunk-pair merged) ----
            xnT = S.tile([128, 2, N], BF16, tag="xnT", name="xnT")
            qT = S.tile([128, 2, NQ], BF16, tag="qT", name="qT")
            kT = S.tile([128, 2, N], BF16, tag="kT", name="kT")
            v2 = S.tile([128, KC, H * (HD + 1)], BF16, tag="v2", name="v2")
            ctx_sb = S.tile([65, H, NQ], BF16, tag="ctx", name="ctx")
            ctxnT = S.tile([128, 2, NQ], BF16, tag="ctxnT", name="ctxnT")
            aoT = S.tile([128, 2, NQ], BF16, tag="aoT", name="aoT")
            xtok = S.tile([128, QT, D], F32, tag="xtok", name="xtok")
            tokl = S.tile([128, QT, D], F32, tag="tokl", name="tokl")
            xn2T = S.tile([128, 2, NQ], BF16, tag="xn2T", name="xn2T")
            hT = S.tile([128, 4, NQ], BF16, tag="hT", name="hT")
            yT = S.tile([128, 2, NQ], BF16, tag="yT", name="yT")
            mv1 = S.tile([128, N // 128, 2], F32, tag="mv1", name="mv1")
            rs1 = S.tile([128, N // 128], F32, tag="rs1", name="rs1")
            nmb = S.tile([128, N // 128], F32, tag="nmb", name="nmb")
            mv2 = S.tile([128, QT, 2], F32, tag="mv2", name="mv2")
            rs2 = S.tile([128, QT], F32, tag="rs2", name="rs2")

            # ---- weights / consts ----
            wq_sb = S.tile([128, 2, D], BF16, tag="wq", name="wq_sb")
            wk_sb = S.tile([128, 2, D], BF16, tag="wk", name="wk_sb")
            wv_sb = S.tile([128, 2, D], BF16, tag="wv", name="wv_sb")
            wo_sb = S.tile([128, 2, D], BF16, tag="wo", name="wo_sb")
            w1_sb = S.tile([128, 2, FF], BF16, tag="w1", name="w1_sb")
            w2_sb = S.tile([128, 4, D], BF16, tag="w2", name="w2_sb")
            for w_sb, w_d in [(wq_sb, wq), (wk_sb, wk), (wv_sb, wv), (wo_sb, wo),
                              (w1_sb, w1), (w2_sb, w2)]:
                nc.sync.dma_start(out=w_sb[:], in_=w_d.rearrange("(c p) d -> p c d", p=128))
            bq_sb = S.tile([128, 2], F32, tag="bq", name="bq_sb")
            bk_sb = S.tile([128, 2], F32, tag="bk", name="bk_sb")
            bva_sb = S.tile([1, H * 65], BF16, tag="bva", name="bva_sb")
            ones1_sb = S.tile([1, 128], BF16, tag="ones1", name="ones1_sb")
            bo_sb = S.tile([128, 2], F32, tag="bo", name="bo_sb")
            b1_sb = S.tile([128, 4], F32, tag="b1", name="b1_sb")
            b2_sb = S.tile([128, 2], F32, tag="b2", name="b2_sb")
            nc.sync.dma_start(out=bq_sb[:], in_=bq[:])
            nc.sync.dma_start(out=bk_sb[:], in_=bk[:])
            nc.sync.dma_start(out=bva_sb[:], in_=bva[:])
            nc.vector.memset(ones1_sb[:], 1.0)
            nc.sync.dma_start(out=bo_sb[:], in_=bo[:])
            nc.sync.dma_start(out=b1_sb[:], in_=b1[:])
            nc.sync.dma_start(out=b2_sb[:], in_=b2[:])
            idb_sb = S.tile([128, 128], BF16, tag="idb", name="idb_sb")
            nc.sync.dma_start(out=idb_sb[:], in_=idb[:])
            imp_sb = S.tile([128, KC], F32, tag="imp", name="imp_sb")
            nc.sync.dma_start(out=imp_sb[:], in_=imp[:])
            eimp_sb = S.tile([128, KC], F32, tag="eimp", name="eimp_sb")
            nc.scalar.activation(out=eimp_sb[:], in_=imp_sb[:], func=EXP)
            eps_sb = S.tile([128, 1], F32, tag="eps", name="eps_sb")
            nc.vector.memset(eps_sb[:], EPS)

            XOR = mybir.AluOpType.bitwise_xor
            SHR = mybir.AluOpType.logical_shift_right
            I32 = mybir.dt.int32

            def quake_rsqrt(var_in, rs_out, n):
                """rs_out[:, :n] = 1/sqrt(var_in + EPS) via DVE-only bit trick."""
                vpe = ST.tile([128, 4], F32, tag="vpe", name="vpe")
                nc.vector.tensor_scalar(out=vpe[:, 0:n], in0=var_in, scalar1=EPS,
                                        scalar2=None, op0=ADD)
                yb = ST.tile([128, 4], I32, tag="yb", name="yb")
                nc.vector.tensor_scalar(out=yb[:, 0:n], in0=vpe[:, 0:n].bitcast(I32),
                                        scalar1=1, scalar2=None, op0=SHR)
                nc.vector.tensor_scalar(out=yb[:, 0:n], in0=yb[:, 0:n], scalar1=-1,
                                        scalar2=None, op0=XOR)
                nc.vector.tensor_scalar(out=yb[:, 0:n], in0=yb[:, 0:n], scalar1=0x5f3759e0,
                                        scalar2=None, op0=ADD)
                y0 = yb[:, 0:n].bitcast(F32)
                t1 = ST.tile([128, 4], F32, tag="t1q", name="t1q")
                y1 = ST.tile([128, 4], F32, tag="y1q", name="y1q")
                nc.vector.tensor_tensor(out=t1[:, 0:n], in0=y0, in1=y0, op=MUL)
                nc.vector.tensor_tensor(out=t1[:, 0:n], in0=t1[:, 0:n], in1=vpe[:, 0:n], op=MUL)
                nc.vector.tensor_scalar(out=t1[:, 0:n], in0=t1[:, 0:n], scalar1=-0.5,
                                        scalar2=1.5, op0=MUL, op1=ADD)
                nc.vector.tensor_tensor(out=y1[:, 0:n], in0=y0, in1=t1[:, 0:n], op=MUL)
                nc.vector.tensor_tensor(out=t1[:, 0:n], in0=y1[:, 0:n], in1=y1[:, 0:n], op=MUL)
                nc.vector.tensor_tensor(out=t1[:, 0:n], in0=t1[:, 0:n], in1=vpe[:, 0:n], op=MUL)
                nc.vector.tensor_scalar(out=t1[:, 0:n], in0=t1[:, 0:n], scalar1=-0.5,
                                        scalar2=1.5, op0=MUL, op1=ADD)
                nc.vector.tensor_tensor(out=rs_out, in0=y1[:, 0:n], in1=t1[:, 0:n], op=MUL)

            # ========= HEAD: LN1 + projections, interleaved per 512-token region =========
            with tc.tile_pool(name="headps", bufs=1, space="PSUM") as HP:
                for r in range(NR):
                    if r < QB:
                        treg = tokl[:, 4 * r:4 * r + 4, :]
                    else:
                        treg = W4.tile([128, 4, D], F32, tag="tokr", name="tokr", bufs=2)[:]
                    nc.sync.dma_start(out=treg,
                                      in_=tok[512 * r:512 * (r + 1), :].rearrange(
                                          "(j p) d -> p j d", p=128))
                    tts = []
                    for j in range(4):
                        i = 4 * r + j
                        tt = treg[:, j, :]
                        tts.append(tt)
                        st = ST.tile([128, 6], F32, tag="st", name="st")
                        nc.vector.bn_stats(out=st[:], in_=tt)
                        nc.vector.bn_aggr(out=mv1[:, i, :], in_=st[:])
                    quake_rsqrt(mv1[:, 4 * r:4 * r + 4, 1], rs1[:, 4 * r:4 * r + 4], 4)
                    nc.vector.tensor_scalar(out=nmb[:, 4 * r:4 * r + 4], in0=mv1[:, 4 * r:4 * r + 4, 0],
                                            scalar1=-1.0, scalar2=None, op0=MUL)
                    nc.vector.tensor_tensor(out=nmb[:, 4 * r:4 * r + 4], in0=nmb[:, 4 * r:4 * r + 4],
                                            in1=rs1[:, 4 * r:4 * r + 4], op=MUL)
                    for j in range(4):
                        i = 4 * r + j
                        xb = W4.tile([128, D], BF16, tag="xnb", name="xnb")
                        nc.scalar.activation(out=xb[:], in_=tts[j],
                                             func=mybir.ActivationFunctionType.Identity,
                                             scale=rs1[:, i:i + 1], bias=nmb[:, i:i + 1])
                        tp = HP.tile([128, 2, 128], BF16, tag="p1t", name="p1t", bufs=2)
                        nc.tensor.transpose(tp[:, 0, :], xb[:, 0:128], idb_sb[:])
                        nc.tensor.transpose(tp[:, 1, :], xb[:, 128:256], idb_sb[:])
                        nc.scalar.copy(out=xnT[:, :, 128 * i:128 * (i + 1)], in_=tp[:])
                    # K projection for this region (bias add on ScalarE)
                    for m in range(2):
                        ps = HP.tile([128, 512], F32, tag="qk", name="kps", bufs=4)
                        for c in range(2):
                            nc.tensor.matmul(ps[:], wk_sb[:, c, 128 * m:128 * (m + 1)],
                                             xnT[:, c, 512 * r:512 * (r + 1)],
                                             start=(c == 0), stop=(c == 1))
                        nc.scalar.add(out=kT[:, m, 512 * r:512 * (r + 1)], in_=ps[:],
                                      add=bk_sb[:, m:m + 1])
                    # Q projection (local queries only)
                    if r < QB:
                        for m in range(2):
                            ps = HP.tile([128, 512], F32, tag="qk", name="qps", bufs=4)
                            for c in range(2):
                                nc.tensor.matmul(ps[:], wq_sb[:, c, 128 * m:128 * (m + 1)],
                                                 xnT[:, c, 512 * r:512 * (r + 1)],
                                                 start=(c == 0), stop=(c == 1))
                            nc.scalar.add(out=qT[:, m, 512 * r:512 * (r + 1)], in_=ps[:],
                                          add=bq_sb[:, m:m + 1])
                    # V'' for this region's key chunks
                    for kc in range(4 * r, 4 * r + 4):
                        ps = HP.tile([128, H * 65], F32, tag="v", name="vps", bufs=2)
                        psr = ps[:].rearrange("p (h j) -> p h j", h=H)
                        for c in range(2):
                            nc.tensor.matmul(psr[:, :, 0:64], xnT[:, c, 128 * kc:128 * (kc + 1)],
                                             wv_sb[:, c, :], start=(c == 0), stop=False,
                                             skip_group_check=True)
                        nc.tensor.matmul(ps[:], ones1_sb[:], bva_sb[:],
                                         start=False, stop=True, skip_group_check=True)
                        nc.vector.tensor_scalar(out=v2[:, kc, :], in0=ps[:],
                                                scalar1=eimp_sb[:, kc:kc + 1], scalar2=None, op0=MUL)

            # ============== ATTENTION + per-qb TAIL (interleaved) ==============
            import collections
            tail_q = collections.deque()

            def drain(k):
                for _ in range(k):
                    if tail_q:
                        tail_q.popleft()()

            with (
                tc.tile_pool(name="p3s", bufs=1, space="PSUM") as P3S,
                tc.tile_pool(name="p3c", bufs=1, space="PSUM") as P3C,
                tc.tile_pool(name="tailps", bufs=1, space="PSUM") as TP,
            ):
                def mk_norm_tr(q, p, t):
                    def f():
                        tp4 = TP.tile([128, 2, 66], BF16, tag="tail", name="tp4")
                        for hp in range(2):
                            nc.tensor.transpose(tp4[:, hp, 0:65],
                                                ctx_sb[0:65, 2 * p + hp, 128 * t:128 * (t + 1)],
                                                idb_sb[0:65, 0:65])
                        rl = ST.tile([128, 2], F32, tag="rl", name="rl")
                        nc.vector.reciprocal(out=rl[:], in_=tp4[:, :, 64:65])
                        ck = W4.tile([128, 128], BF16, tag="ck", name="ck")
                        for hp in range(2):
                            nc.vector.tensor_scalar(out=ck[:, 64 * hp:64 * (hp + 1)],
                                                    in0=tp4[:, hp, 0:64],
                                                    scalar1=rl[:, hp:hp + 1], scalar2=None, op0=MUL)
                        tb = TP.tile([128, 128], BF16, tag="tail", name="tb")
                        nc.tensor.transpose(tb[:], ck[:], idb_sb[:])
                        nc.vector.tensor_copy(out=ctxnT[:, p, 128 * t:128 * (t + 1)], in_=tb[:])
                    return f

                def mk_wo(q, m):
                    def f():
                        ps = TP.tile([128, 512], F32, tag="tail", name="wops")
                        for c in range(2):
                            nc.tensor.matmul(ps[:], wo_sb[:, c, 128 * m:128 * (m + 1)],
                                             ctxnT[:, c, 512 * q:512 * (q + 1)],
                                             start=(c == 0), stop=(c == 1), skip_group_check=True)
                        nc.vector.tensor_scalar(out=aoT[:, m, 512 * q:512 * (q + 1)], in0=ps[:],
                                                scalar1=bo_sb[:, m:m + 1], scalar2=None, op0=ADD)
                    return f

                def mk_resid(q, t):
                    def f():
                        tb = TP.tile([128, 2, 128], BF16, tag="tail", name="aot")
                        nc.tensor.transpose(tb[:, 0, :], aoT[:, 0, 128 * t:128 * (t + 1)], idb_sb[:])
                        nc.tensor.transpose(tb[:, 1, :], aoT[:, 1, 128 * t:128 * (t + 1)], idb_sb[:])
                        nc.vector.tensor_tensor(out=xtok[:, t, :], in0=tb.rearrange("p a b -> p (a b)"),
                                                in1=tokl[:, t, :], op=ADD)
                        st = ST.tile([128, 6], F32, tag="st", name="st")
                        nc.vector.bn_stats(out=st[:], in_=xtok[:, t, :])
                        nc.vector.bn_aggr(out=mv2[:, t, :], in_=st[:])
                    return f

                def mk_rstd2(q):
                    def f():
                        quake_rsqrt(mv2[:, 4 * q:4 * q + 4, 1], rs2[:, 4 * q:4 * q + 4], 4)
                    return f

                def mk_ln2(q, t):
                    def f():
                        xb = W4.tile([128, D], BF16, tag="xnb", name="xnb")
                        nc.vector.tensor_scalar(out=xb[:], in0=xtok[:, t, :], scalar1=mv2[:, t, 0:1],
                                                scalar2=rs2[:, t:t + 1], op0=SUB, op1=MUL)
                        tp2 = TP.tile([128, 2, 128], BF16, tag="tail", name="p8t")
                        nc.tensor.transpose(tp2[:, 0, :], xb[:, 0:128], idb_sb[:])
                        nc.tensor.transpose(tp2[:, 1, :], xb[:, 128:256], idb_sb[:])
                        nc.vector.tensor_copy(out=xn2T[:, :, 128 * t:128 * (t + 1)], in_=tp2[:])
                    return f


                def mk_ffn2(q, m):
                    def f():
                        ps = TP.tile([128, 512], F32, tag="tail", name="yps")
                        for c in range(4):
                            nc.tensor.matmul(ps[:], w2_sb[:, c, 128 * m:128 * (m + 1)],
                                             hT[:, c, 512 * q:512 * (q + 1)],
                                             start=(c == 0), stop=(c == 3), skip_group_check=True)
                        nc.vector.tensor_scalar(out=yT[:, m, 512 * q:512 * (q + 1)], in0=ps[:],
                                                scalar1=b2_sb[:, m:m + 1], scalar2=None, op0=ADD)
                    return f

                def mk_out(q, t):
                    def f():
                        tb = TP.tile([128, 2, 128], BF16, tag="tail", name="yt")
                        nc.tensor.transpose(tb[:, 0, :], yT[:, 0, 128 * t:128 * (t + 1)], idb_sb[:])
                        nc.tensor.transpose(tb[:, 1, :], yT[:, 1, 128 * t:128 * (t + 1)], idb_sb[:])
                        ot = W4.tile([128, D], F32, tag="ot", name="ot")
                        nc.vector.tensor_tensor(out=ot[:], in0=tb.rearrange("p a b -> p (a b)"),
                                                in1=xtok[:, t, :], op=ADD)
                        nc.sync.dma_start(out=out[128 * t:128 * (t + 1), :], in_=ot[:])
                    return f

                for q in range(QB):
                    for p in range(2):
                        cps = P3C.tile([65, 2, 512], F32, tag="ctxps", name="ctxps")
                        slots = [(kc, hp) for kc in range(KC) for hp in range(2)]
                        sizes = [3, 2] * 12 + [3, 1]
                        g = 0
                        pending = []

                        def emit_ctx(pend):
                            pt_, slots_ = pend
                            for j_, (kc_, hp_) in enumerate(slots_):
                                h_ = 2 * p + hp_
                                nc.tensor.matmul(cps[:, hp_, :], v2[:, kc_, 65 * h_:65 * (h_ + 1)],
                                                 pt_[:, j_, :], start=(kc_ == 0),
                                                 stop=(kc_ == KC - 1), skip_group_check=True)

                        for n in sizes:
                            tagn = "sgA" if n == 3 else "sgB"
                            sg = P3S.tile([128, 3 if n == 3 else 2, 512], F32,
                                          tag=tagn, name=tagn)
                            for j in range(n):
                                kc, hp = slots[g + j]
                                nc.tensor.matmul(
                                    sg[:, j, :],
                                    kT[64 * hp:64 * (hp + 1), p, 128 * kc:128 * (kc + 1)],
                                    qT[64 * hp:64 * (hp + 1), p, 512 * q:512 * (q + 1)],
                                    start=True, stop=True, skip_group_check=True)
                            pt = W4.tile([128, 3 if n == 3 else 2, 512], BF16,
                                         tag=f"pt{tagn}", name=f"pt{tagn}", bufs=4)
                            nc.scalar.activation(out=pt[:, 0:n, :], in_=sg[:, 0:n, :],
                                                 func=EXP, scale=0.125)
                            pending.append((pt, [slots[g + j] for j in range(n)]))
                            if len(pending) > 4:
                                emit_ctx(pending.pop(0))
                            g += n
                            drain(1)
                        for pend in pending:
                            emit_ctx(pend)
                        for hp in range(2):
                            nc.vector.tensor_copy(out=ctx_sb[:, 2 * p + hp, 512 * q:512 * (q + 1)],
                                                  in_=cps[:, hp, :])
                        for t in range(4 * q, 4 * q + 4):
                            tail_q.append(mk_norm_tr(q, p, t))
                    for m in range(2):
                        tail_q.append(mk_wo(q, m))
                    for t in range(4 * q, 4 * q + 4):
                        tail_q.append(mk_resid(q, t))
                    tail_q.append(mk_rstd2(q))
                    for t in range(4 * q, 4 * q + 4):
                        tail_q.append(mk_ln2(q, t))
                drain(len(tail_q))

            # ============== gelu + FFN2 + output (post-attention) ==============
            with tc.tile_pool(name="ffps", bufs=1, space="PSUM") as FP:
                for q in range(QB):
                    for f_ in range(4):
                        ps = FP.tile([128, 512], F32, tag="ff", name="ffps2", bufs=4)
                        for c in range(2):
                            nc.tensor.matmul(ps[:], w1_sb[:, c, 128 * f_:128 * (f_ + 1)],
                                             xn2T[:, c, 512 * q:512 * (q + 1)],
                                             start=(c == 0), stop=(c == 1), skip_group_check=True)
                        nc.scalar.activation(out=hT[:, f_, 512 * q:512 * (q + 1)], in_=ps[:],
                                             func=GELU, bias=b1_sb[:, f_:f_ + 1], scale=1.0)
                    for m in range(2):
                        ps = FP.tile([128, 512], F32, tag="y2", name="yps2", bufs=2)
                        for c in range(4):
                            nc.tensor.matmul(ps[:], w2_sb[:, c, 128 * m:128 * (m + 1)],
                                             hT[:, c, 512 * q:512 * (q + 1)],
                                             start=(c == 0), stop=(c == 3), skip_group_check=True)
                        nc.vector.tensor_scalar(out=yT[:, m, 512 * q:512 * (q + 1)], in0=ps[:],
                                                scalar1=b2_sb[:, m:m + 1], scalar2=None, op0=ADD)
                    for t in range(4 * q, 4 * q + 4):
                        tb = FP.tile([128, 2, 128], BF16, tag="ytr", name="yt2", bufs=2)
                        nc.tensor.transpose(tb[:, 0, :], yT[:, 0, 128 * t:128 * (t + 1)], idb_sb[:])
                        nc.tensor.transpose(tb[:, 1, :], yT[:, 1, 128 * t:128 * (t + 1)], idb_sb[:])
                        ot = W4.tile([128, D], F32, tag="ot", name="ot")
                        nc.vector.tensor_tensor(out=ot[:], in0=tb.rearrange("p a b -> p (a b)"),
                                                in1=xtok[:, t, :], op=ADD)
                        nc.sync.dma_start(out=out[128 * t:128 * (t + 1), :], in_=ot[:])

    nc.compile()
    return nc


def _get_compiled():
    global _compiled
    if _compiled is None:
        _compiled = _build()
    return _compiled


def _bva(bv_f):
    a = np.ones((1, H * (HD + 1)), np.float32)
    for h in range(H):
        a[0, 65 * h:65 * h + 64] = bv_f[64 * h:64 * (h + 1)]
    return a.astype(BF)


def _prep_in_maps(tokens, importance, norm1_w, norm1_b, Wq, bq, Wk, bk, Wv, bv,
                  Wo, bo, norm2_w, norm2_b, W1, b1, W2, b2):
    f32 = np.float32
    tokens = np.asarray(tokens, f32)
    importance = np.asarray(importance, f32)

    # fold LN affine params into the following projection weights
    Wq_f = (np.asarray(norm1_w, f32)[:, None] * np.asarray(Wq, f32))
    Wk_f = (np.asarray(norm1_w, f32)[:, None] * np.asarray(Wk, f32))
    Wv_f = (np.asarray(norm1_w, f32)[:, None] * np.asarray(Wv, f32))
    bq_f = np.asarray(norm1_b, f32) @ np.asarray(Wq, f32) + np.asarray(bq, f32)
    bk_f = np.asarray(norm1_b, f32) @ np.asarray(Wk, f32) + np.asarray(bk, f32)
    bv_f = np.asarray(norm1_b, f32) @ np.asarray(Wv, f32) + np.asarray(bv, f32)
    W1_f = (np.asarray(norm2_w, f32)[:, None] * np.asarray(W1, f32))
    b1_f = np.asarray(norm2_b, f32) @ np.asarray(W1, f32) + np.asarray(b1, f32)

    common = {
        "wq": Wq_f.astype(BF), "wk": Wk_f.astype(BF), "wv": Wv_f.astype(BF),
        "wo": np.asarray(Wo, f32).astype(BF),
        "w1": W1_f.astype(BF), "w2": np.asarray(W2, f32).astype(BF),
        "bq": np.ascontiguousarray(bq_f.reshape(2, 128).T.astype(f32)),
        "bk": np.ascontiguousarray(bk_f.reshape(2, 128).T.astype(f32)),
        "bva": _bva(bv_f),
        "bo": np.ascontiguousarray(np.asarray(bo, f32).reshape(2, 128).T),
        "b1": np.ascontiguousarray(b1_f.reshape(4, 128).T.astype(f32)),
        "b2": np.ascontiguousarray(np.asarray(b2, f32).reshape(2, 128).T),
        "idb": np.eye(128, dtype=f32).astype(BF),
    }

    in_maps = []
    for c in range(8):
        b = c // 2
        qh = c % 2
        qs = qh * NQ
        perm = np.r_[qs:qs + NQ, (0 if qh else NQ):(NQ if qh else N)]
        toks = np.ascontiguousarray(tokens[b][perm])
        impp = np.ascontiguousarray(importance[b][perm].reshape(KC, 128).T.astype(f32))
        in_maps.append({"tokens": toks, "imp": impp, **common})
    return in_maps


def _run(in_maps, trace=False):
    nc = _get_compiled()
    return run_bass_kernel_spmd(nc, in_maps, core_ids=list(range(8)), trace=trace)


def _assemble(res):
    out = np.empty((B, N, D), np.float32)
    for c in range(8):
        b = c // 2
        qs = (c % 2) * NQ
        out[b, qs:qs + NQ] = res.results[c]["out"]
    return out


def kernel(**inputs) -> np.ndarray:
    res = _run(_prep_in_maps(**inputs), trace=False)
    return _assemble(res)


def kernel_traced(**inputs):
    """Like kernel() but with NTFF profiling; returns (out, exec_time_ns)."""
    res = _run(_prep_in_maps(**inputs), trace=True)
    return _assemble(res), res.exec_time_ns


# revision 21
# speedup vs baseline: 1.0106x; 1.0106x over previous
"""Trainium2 Bass kernel for nn_AGSISpaBlock (pre-norm MHA + GELU FFN block).

Sharding: 8 cores; core c handles batch b = c//2 and query-half qh = c%2.
Each core receives its batch's tokens PERMUTED so its 2048 local query rows
come first (attention is permutation-invariant over keys, so one SPMD graph
serves all cores). No collectives needed.

Dataflow on each core (all matmuls bf16 with fp32 PSUM accumulation):
  head:  LN1 (token-major, bn_stats, per-region batched sqrt) -> xn^T via PE
         transposes; Q^T/K^T projections; V token-major with importance
         folded in as exp(importance) row scaling (V'' = eimp * [V | 1]).
         LN1 and projections interleaved per 512-token region.
  attn:  keys-major scores S^T[k, q] via head-pair row-packed matmuls
         (heads 2p/2p+1 on PE rows 0-63/64-127 concurrently); exp(S/8) on
         ScalarE straight out of PSUM in 3/2-bank alternating groups;
         ctx^T[hd+1, q] accumulated over key chunks (ones column of V''
         carries the softmax denominator L).  qb-outer / pair-inner loop so
         each 512-query block finishes attention early.
  tail:  per qb: transpose ctx to token-major (L becomes a per-partition
         column) -> 1/L normalize -> transpose back -> Wo -> residual ->
         LN2 -> FFN (exact-erf Gelu) -> final residual -> DMA out.  All
         tail PSUM work goes through one reserved bank so it overlaps the
         attention of later query blocks.
"""

import sys

if "/opt/trn_rl_repo" not in sys.path:
    sys.path.insert(0, "/opt/trn_rl_repo")

import numpy as np
import ml_dtypes

import concourse.bass as bass
import concourse.tile as tile
from concourse import bacc, mybir
from concourse.bass_utils import run_bass_kernel_spmd

F32 = mybir.dt.float32
BF16 = mybir.dt.bfloat16
BF = ml_dtypes.bfloat16

B, N, D = 4, 4096, 256
H, HD = 4, 64
FF = 512
EPS = 1e-5
NQ = N // 2          # local queries per core
KC = N // 128        # key chunks (32)
QB = NQ // 512       # 512-wide query blocks (4)
QT = NQ // 128       # 128-wide query tiles (16)
NR = N // 512        # 512-token regions (8)

_compiled = None


def _build():
    nc = bacc.Bacc("TRN2", target_bir_lowering=False, debug=False, num_devices=8)

    tok = nc.declare_dram_parameter("tokens", [N, D], F32, isOutput=False)
    imp = nc.declare_dram_parameter("imp", [128, KC], F32, isOutput=False)
    wq = nc.declare_dram_parameter("wq", [D, D], BF16, isOutput=False)
    wk = nc.declare_dram_parameter("wk", [D, D], BF16, isOutput=False)
    wv = nc.declare_dram_parameter("wv", [D, D], BF16, isOutput=False)
    wo = nc.declare_dram_parameter("wo", [D, D], BF16, isOutput=False)
    w1 = nc.declare_dram_parameter("w1", [D, FF], BF16, isOutput=False)
    w2 = nc.declare_dram_parameter("w2", [FF, D], BF16, isOutput=False)
    bq = nc.declare_dram_parameter("bq", [128, 2], F32, isOutput=False)
    bk = nc.declare_dram_parameter("bk", [128, 2], F32, isOutput=False)
    bva = nc.declare_dram_parameter("bva", [1, H * 65], BF16, isOutput=False)
    bo = nc.declare_dram_parameter("bo", [128, 2], F32, isOutput=False)
    b1 = nc.declare_dram_parameter("b1", [128, 4], F32, isOutput=False)
    b2 = nc.declare_dram_parameter("b2", [128, 2], F32, isOutput=False)
    idb = nc.declare_dram_parameter("idb", [128, 128], BF16, isOutput=False)
    out = nc.declare_dram_parameter("out", [NQ, D], F32, isOutput=True)

    EXP = mybir.ActivationFunctionType.Exp
    GELU = mybir.ActivationFunctionType.Gelu
    SQRT = mybir.ActivationFunctionType.Sqrt
    SUB = mybir.AluOpType.subtract
    MUL = mybir.AluOpType.mult
    ADD = mybir.AluOpType.add

    with tile.TileContext(nc) as tc:
        with (
            tc.tile_pool(name="singles", bufs=1) as S,
            tc.tile_pool(name="work", bufs=4) as W4,
            tc.tile_pool(name="stats", bufs=4) as ST,
        ):
            # ---- persistent SBUF tensors (chunk-pair merged) ----
            xnT = S.tile([128, 2, N], BF16, tag="xnT", name="xnT")
            qT = S.tile([128, 2, NQ], BF16, tag="qT", name="qT")
            kT = S.tile([128, 2, N], BF16, tag="kT", name="kT")
            v2 = S.tile([128, KC, H * (HD + 1)], BF16, tag="v2", name="v2")
            ctx_sb = S.tile([65, H, NQ], BF16, tag="ctx", name="ctx")
            ctxnT = S.tile([128, 2, NQ], BF16, tag="ctxnT", name="ctxnT")
            aoT = S.tile([128, 2, NQ], BF16, tag="aoT", name="aoT")
            xtok = S.tile([128, QT, D], F32, tag="xtok", name="xtok")
            tokl = S.tile([128, QT, D], F32, tag="tokl", name="tokl")
            xn2T = S.tile([128, 2, NQ], BF16, tag="xn2T", name="xn2T")
            hT = S.tile([128, 4, NQ], BF16, tag="hT", name="hT")
            yT = S.tile([128, 2, NQ], BF16, tag="yT", name="yT")
            mv1 = S.tile([128, N // 128, 2], F32, tag="mv1", name="mv1")
            rs1 = S.tile([128, N // 128], F32, tag="rs1", name="rs1")
            nmb = S.tile([128, N // 128], F32, tag="nmb", name="nmb")
            mv2 = S.tile([128, QT, 2], F32, tag="mv2", name="mv2")
            rs2 = S.tile([128, QT], F32, tag="rs2", name="rs2")

            # ---- weights / consts ----
            wq_sb = S.tile([128, 2, D], BF16, tag="wq", name="wq_sb")
            wk_sb = S.tile([128, 2, D], BF16, tag="wk", name="wk_sb")
            wv_sb = S.tile([128, 2, D], BF16, tag="wv", name="wv_sb")
            wo_sb = S.tile([128, 2, D], BF16, tag="wo", name="wo_sb")
            w1_sb = S.tile([128, 2, FF], BF16, tag="w1", name="w1_sb")
            w2_sb = S.tile([128, 4, D], BF16, tag="w2", name="w2_sb")
            for w_sb, w_d in [(wq_sb, wq), (wk_sb, wk), (wv_sb, wv), (wo_sb, wo),
                              (w1_sb, w1), (w2_sb, w2)]:
                nc.sync.dma_start(out=w_sb[:], in_=w_d.rearrange("(c p) d -> p c d", p=128))
            bq_sb = S.tile([128, 2], F32, tag="bq", name="bq_sb")
            bk_sb = S.tile([128, 2], F32, tag="bk", name="bk_sb")
            bva_sb = S.tile([1, H * 65], BF16, tag="bva", name="bva_sb")
            ones1_sb = S.tile([1, 128], BF16, tag="ones1", name="ones1_sb")
            bo_sb = S.tile([128, 2], F32, tag="bo", name="bo_sb")
            b1_sb = S.tile([128, 4], F32, tag="b1", name="b1_sb")
            b2_sb = S.tile([128, 2], F32, tag="b2", name="b2_sb")
            nc.sync.dma_start(out=bq_sb[:], in_=bq[:])
            nc.sync.dma_start(out=bk_sb[:], in_=bk[:])
            nc.sync.dma_start(out=bva_sb[:], in_=bva[:])
            nc.vector.memset(ones1_sb[:], 1.0)
            nc.sync.dma_start(out=bo_sb[:], in_=bo[:])
            nc.sync.dma_start(out=b1_sb[:], in_=b1[:])
            nc.sync.dma_start(out=b2_sb[:], in_=b2[:])
            idb_sb = S.tile([128, 128], BF16, tag="idb", name="idb_sb")
            nc.sync.dma_start(out=idb_sb[:], in_=idb[:])
            imp_sb = S.tile([128, KC], F32, tag="imp", name="imp_sb")
            nc.sync.dma_start(out=imp_sb[:], in_=imp[:])
            eimp_sb = S.tile([128, KC], F32, tag="eimp", name="eimp_sb")
            nc.scalar.activation(out=eimp_sb[:], in_=imp_sb[:], func=EXP)
            eps_sb = S.tile([128, 1], F32, tag="eps", name="eps_sb")
            nc.vector.memset(eps_sb[:], EPS)

            XOR = mybir.AluOpType.bitwise_xor
            SHR = mybir.AluOpType.logical_shift_right
            I32 = mybir.dt.int32

            def quake_rsqrt(var_in, rs_out, n):
                """rs_out[:, :n] = 1/sqrt(var_in + EPS) via DVE-only bit trick."""
                vpe = ST.tile([128, 4], F32, tag="vpe", name="vpe")
                nc.vector.tensor_scalar(out=vpe[:, 0:n], in0=var_in, scalar1=EPS,
                                        scalar2=None, op0=ADD)
                yb = ST.tile([128, 4], I32, tag="yb", name="yb")
                nc.vector.tensor_scalar(out=yb[:, 0:n], in0=vpe[:, 0:n].bitcast(I32),
                                        scalar1=1, scalar2=None, op0=SHR)
                nc.vector.tensor_scalar(out=yb[:, 0:n], in0=yb[:, 0:n], scalar1=-1,
                                        scalar2=None, op0=XOR)
                nc.vector.tensor_scalar(out=yb[:, 0:n], in0=yb[:, 0:n], scalar1=0x5f3759e0,
                                        scalar2=None, op0=ADD)
                y0 = yb[:, 0:n].bitcast(F32)
                t1 = ST.tile([128, 4], F32, tag="t1q", name="t1q")
                y1 = ST.tile([128, 4], F32, tag="y1q", name="y1q")
                nc.vector.tensor_tensor(out=t1[:, 0:n], in0=y0, in1=y0, op=MUL)
                nc.vector.tensor_tensor(out=t1[:, 0:n], in0=t1[:, 0:n], in1=vpe[:, 0:n], op=MUL)
                nc.vector.tensor_scalar(out=t1[:, 0:n], in0=t1[:, 0:n], scalar1=-0.5,
                                        scalar2=1.5, op0=MUL, op1=ADD)
                nc.vector.tensor_tensor(out=y1[:, 0:n], in0=y0, in1=t1[:, 0:n], op=MUL)
                nc.vector.tensor_tensor(out=t1[:, 0:n], in0=y1[:, 0:n], in1=y1[:, 0:n], op=MUL)
                nc.vector.tensor_tensor(out=t1[:, 0:n], in0=t1[:, 0:n], in1=vpe[:, 0:n], op=MUL)
                nc.vector.tensor_scalar(out=t1[:, 0:n], in0=t1[:, 0:n], scalar1=-0.5,
                                        scalar2=1.5, op0=MUL, op1=ADD)
                nc.vector.tensor_tensor(out=rs_out, in0=y1[:, 0:n], in1=t1[:, 0:n], op=MUL)

            # ========= HEAD: LN1 + projections, interleaved per 512-token region =========
            with tc.tile_pool(name="headps", bufs=1, space="PSUM") as HP:
                for r in range(NR):
                    if r < QB:
                        treg = tokl[:, 4 * r:4 * r + 4, :]
                    else:
                        treg = W4.tile([128, 4, D], F32, tag="tokr", name="tokr", bufs=2)[:]
                    nc.sync.dma_start(out=treg,
                                      in_=tok[512 * r:512 * (r + 1), :].rearrange(
                                          "(j p) d -> p j d", p=128))
                    tts = []
                    for j in range(4):
                        i = 4 * r + j
                        tt = treg[:, j, :]
                        tts.append(tt)
                        st = ST.tile([128, 6], F32, tag="st", name="st")
                        nc.vector.bn_stats(out=st[:], in_=tt)
                        nc.vector.bn_aggr(out=mv1[:, i, :], in_=st[:])
                    quake_rsqrt(mv1[:, 4 * r:4 * r + 4, 1], rs1[:, 4 * r:4 * r + 4], 4)
                    nc.vector.tensor_scalar(out=nmb[:, 4 * r:4 * r + 4], in0=mv1[:, 4 * r:4 * r + 4, 0],
                                            scalar1=-1.0, scalar2=None, op0=MUL)
                    nc.vector.tensor_tensor(out=nmb[:, 4 * r:4 * r + 4], in0=nmb[:, 4 * r:4 * r + 4],
                                            in1=rs1[:, 4 * r:4 * r + 4], op=MUL)
                    for j in range(4):
                        i = 4 * r + j
                        xb = W4.tile([128, D], BF16, tag="xnb", name="xnb")
                        nc.scalar.activation(out=xb[:], in_=tts[j],
                                             func=mybir.ActivationFunctionType.Identity,
                                             scale=rs1[:, i:i + 1], bias=nmb[:, i:i + 1])
                        tp = HP.tile([128, 2, 128], BF16, tag="p1t", name="p1t", bufs=2)
                        nc.tensor.transpose(tp[:, 0, :], xb[:, 0:128], idb_sb[:])
                        nc.tensor.transpose(tp[:, 1, :], xb[:, 128:256], idb_sb[:])
                        nc.vector.tensor_copy(out=xnT[:, :, 128 * i:128 * (i + 1)], in_=tp[:])
                    # K projection for this region (bias add on ScalarE)
                    for m in range(2):
                        ps = HP.tile([128, 512], F32, tag="qk", name="kps", bufs=4)
                        for c in range(2):
                            nc.tensor.matmul(ps[:], wk_sb[:, c, 128 * m:128 * (m + 1)],
                                             xnT[:, c, 512 * r:512 * (r + 1)],
                                             start=(c == 0), stop=(c == 1))
                        nc.scalar.add(out=kT[:, m, 512 * r:512 * (r + 1)], in_=ps[:],
                                      add=bk_sb[:, m:m + 1])
                    # Q projection (local queries only)
                    if r < QB:
                        for m in range(2):
                            ps = HP.tile([128, 512], F32, tag="qk", name="qps", bufs=4)
                            for c in range(2):
                                nc.tensor.matmul(ps[:], wq_sb[:, c, 128 * m:128 * (m + 1)],
                                                 xnT[:, c, 512 * r:512 * (r + 1)],
                                                 start=(c == 0), stop=(c == 1))
                            nc.scalar.add(out=qT[:, m, 512 * r:512 * (r + 1)], in_=ps[:],
                                          add=bq_sb[:, m:m + 1])
                    # V'' for this region's key chunks
                    for kc in range(4 * r, 4 * r + 4):
                        ps = HP.tile([128, H * 65], F32, tag="v", name="vps", bufs=2)
                        psr = ps[:].rearrange("p (h j) -> p h j", h=H)
                        for c in range(2):
                            nc.tensor.matmul(psr[:, :, 0:64], xnT[:, c, 128 * kc:128 * (kc + 1)],
                                             wv_sb[:, c, :], start=(c == 0), stop=False,
                                             skip_group_check=True)
                        nc.tensor.matmul(ps[:], ones1_sb[:], bva_sb[:],
                                         start=False, stop=True, skip_group_check=True)
                        nc.vector.tensor_scalar(out=v2[:, kc, :], in0=ps[:],
                                                scalar1=eimp_sb[:, kc:kc + 1], scalar2=None, op0=MUL)

            # ============== ATTENTION + per-qb TAIL (interleaved) ==============
            import collections
            tail_q = collections.deque()

            def drain(k):
                for _ in range(k):
                    if tail_q:
                        tail_q.popleft()()

            with (
                tc.tile_pool(name="p3s", bufs=1, space="PSUM") as P3S,
                tc.tile_pool(name="p3c", bufs=1, space="PSUM") as P3C,
                tc.tile_pool(name="tailps", bufs=1, space="PSUM") as TP,
            ):
                def mk_norm_tr(q, p, t):
                    def f():
                        tp4 = TP.tile([128, 2, 66], BF16, tag="tail", name="tp4")
                        for hp in range(2):
                            nc.tensor.transpose(tp4[:, hp, 0:65],
                                                ctx_sb[0:65, 2 * p + hp, 128 * t:128 * (t + 1)],
                                                idb_sb[0:65, 0:65])
                        rl = ST.tile([128, 2], F32, tag="rl", name="rl")
                        nc.vector.reciprocal(out=rl[:], in_=tp4[:, :, 64:65])
                        ck = W4.tile([128, 128], BF16, tag="ck", name="ck")
                        for hp in range(2):
                            nc.vector.tensor_scalar(out=ck[:, 64 * hp:64 * (hp + 1)],
                                                    in0=tp4[:, hp, 0:64],
                                                    scalar1=rl[:, hp:hp + 1], scalar2=None, op0=MUL)
                        tb = TP.tile([128, 128], BF16, tag="tail", name="tb")
                        nc.tensor.transpose(tb[:], ck[:], idb_sb[:])
                        nc.vector.tensor_copy(out=ctxnT[:, p, 128 * t:128 * (t + 1)], in_=tb[:])
                    return f

                def mk_wo(q, m):
                    def f():
                        ps = TP.tile([128, 512], F32, tag="tail", name="wops")
                        for c in range(2):
                            nc.tensor.matmul(ps[:], wo_sb[:, c, 128 * m:128 * (m + 1)],
                                             ctxnT[:, c, 512 * q:512 * (q + 1)],
                                             start=(c == 0), stop=(c == 1), skip_group_check=True)
                        nc.vector.tensor_scalar(out=aoT[:, m, 512 * q:512 * (q + 1)], in0=ps[:],
                                                scalar1=bo_sb[:, m:m + 1], scalar2=None, op0=ADD)
                    return f

                def mk_resid(q, t):
                    def f():
                        tb = TP.tile([128, 2, 128], BF16, tag="tail", name="aot")
                        nc.tensor.transpose(tb[:, 0, :], aoT[:, 0, 128 * t:128 * (t + 1)], idb_sb[:])
                        nc.tensor.transpose(tb[:, 1, :], aoT[:, 1, 128 * t:128 * (t + 1)], idb_sb[:])
                        nc.vector.tensor_tensor(out=xtok[:, t, :], in0=tb.rearrange("p a b -> p (a b)"),
                                                in1=tokl[:, t, :], op=ADD)
                        st = ST.tile([128, 6], F32, tag="st", name="st")
                        nc.vector.bn_stats(out=st[:], in_=xtok[:, t, :])
                        nc.vector.bn_aggr(out=mv2[:, t, :], in_=st[:])
                    return f

                def mk_rstd2(q):
                    def f():
                        quake_rsqrt(mv2[:, 4 * q:4 * q + 4, 1], rs2[:, 4 * q:4 * q + 4], 4)
                    return f

                def mk_ln2(q, t):
                    def f():
                        xb = W4.tile([128, D], BF16, tag="xnb", name="xnb")
                        nc.vector.tensor_scalar(out=xb[:], in0=xtok[:, t, :], scalar1=mv2[:, t, 0:1],
                                                scalar2=rs2[:, t:t + 1], op0=SUB, op1=MUL)
                        tp2 = TP.tile([128, 2, 128], BF16, tag="tail", name="p8t")
                        nc.tensor.transpose(tp2[:, 0, :], xb[:, 0:128], idb_sb[:])
                        nc.tensor.transpose(tp2[:, 1, :], xb[:, 128:256], idb_sb[:])
                        nc.vector.tensor_copy(out=xn2T[:, :, 128 * t:128 * (t + 1)], in_=tp2[:])
                    return f


                def mk_ffn2(q, m):
                    def f():
                        ps = TP.tile([128, 512], F32, tag="tail", name="yps")
                        for c in range(4):
                            nc.tensor.matmul(ps[:], w2_sb[:, c, 128 * m:128 * (m + 1)],
                                             hT[:, c, 512 * q:512 * (q + 1)],
                                             start=(c == 0), stop=(c == 3), skip_group_check=True)
                        nc.vector.tensor_scalar(out=yT[:, m, 512 * q:512 * (q + 1)], in0=ps[:],
                                                scalar1=b2_sb[:, m:m + 1], scalar2=None, op0=ADD)
                    return f

                def mk_out(q, t):
                    def f():
                        tb = TP.tile([128, 2, 128], BF16, tag="tail", name="yt")
                        nc.tensor.transpose(tb[:, 0, :], yT[:, 0, 128 * t:128 * (t + 1)], idb_sb[:])
                        nc.tensor.transpose(tb[:, 1, :], yT[:, 1, 128 * t:128 * (t + 1)], idb_sb[:])
                        ot = W4.tile([128, D], F32, tag="ot", name="ot")
                        nc.vector.tensor_tensor(out=ot[:], in0=tb.rearrange("p a b -> p (a b)"),
                                                in1=xtok[:, t, :], op=ADD)
                        nc.sync.dma_start(out=out[128 * t:128 * (t + 1), :], in_=ot[:])
                    return f

                for q in range(QB):
                    for p in range(2):
                        cps = P3C.tile([65, 2, 512], F32, tag="ctxps", name="ctxps")
                        slots = [(kc, hp) for kc in range(KC) for hp in range(2)]
                        sizes = [3, 2] * 12 + [3, 1]
                        g = 0
                        pending = []

                        def emit_ctx(pend):
                            pt_, slots_ = pend
                            for j_, (kc_, hp_) in enumerate(slots_):
                                h_ = 2 * p + hp_
                                nc.tensor.matmul(cps[:, hp_, :], v2[:, kc_, 65 * h_:65 * (h_ + 1)],
                                                 pt_[:, j_, :], start=(kc_ == 0),
                                                 stop=(kc_ == KC - 1), skip_group_check=True)

                        for n in sizes:
                            tagn = "sgA" if n == 3 else "sgB"
                            sg = P3S.tile([128, 3 if n == 3 else 2, 512], F32,
                                          tag=tagn, name=tagn)
                            for j in range(n):
                                kc, hp = slots[g + j]
                                nc.tensor.matmul(
                                    sg[:, j, :],
                                    kT[64 * hp:64 * (hp + 1), p, 128 * kc:128 * (kc + 1)],
                                    qT[64 * hp:64 * (hp + 1), p, 512 * q:512 * (q + 1)],
                                    start=True, stop=True, skip_group_check=True)
                            pt = W4.tile([128, 3 if n == 3 else 2, 512], BF16,
                                         tag=f"pt{tagn}", name=f"pt{tagn}", bufs=4)
                            nc.scalar.activation(out=pt[:, 0:n, :], in_=sg[:, 0:n, :],
                                                 func=EXP, scale=0.125)
                            pending.append((pt, [slots[g + j] for j in range(n)]))
                            if len(pending) > 4:
                                emit_ctx(pending.pop(0))
                            g += n
                            drain(1)
                        for pend in pending:
                            emit_ctx(pend)
                        for hp in range(2):
                            nc.vector.tensor_copy(out=ctx_sb[:, 2 * p + hp, 512 * q:512 * (q + 1)],
                                                  in_=cps[:, hp, :])
                        for t in range(4 * q, 4 * q + 4):
                            tail_q.append(mk_norm_tr(q, p, t))
                    for m in range(2):
                        tail_q.append(mk_wo(q, m))
                    for t in range(4 * q, 4 * q + 4):
                        tail_q.append(mk_resid(q, t))
                    tail_q.append(mk_rstd2(q))
                    for t in range(4 * q, 4 * q + 4):
                        tail_q.append(mk_ln2(q, t))
                drain(len(tail_q))

            # ============== gelu + FFN2 + output (post-attention) ==============
            with tc.tile_pool(name="ffps", bufs=1, space="PSUM") as FP:
                for q in range(QB):
                    for f_ in range(4):
                        ps = FP.tile([128, 512], F32, tag="ff", name="ffps2", bufs=4)
                        for c in range(2):
                            nc.tensor.matmul(ps[:], w1_sb[:, c, 128 * f_:128 * (f_ + 1)],
                                             xn2T[:, c, 512 * q:512 * (q + 1)],
                                             start=(c == 0), stop=(c == 1), skip_group_check=True)
                        nc.scalar.activation(out=hT[:, f_, 512 * q:512 * (q + 1)], in_=ps[:],
                                             func=GELU, bias=b1_sb[:, f_:f_ + 1], scale=1.0)
                    for m in range(2):
                        ps = FP.tile([128, 512], F32, tag="y2", name="yps2", bufs=2)
                        for c in range(4):
                            nc.tensor.matmul(ps[:], w2_sb[:, c, 128 * m:128 * (m + 1)],
                                             hT[:, c, 512 * q:512 * (q + 1)],
                                             start=(c == 0), stop=(c == 3), skip_group_check=True)
                        nc.vector.tensor_scalar(out=yT[:, m, 512 * q:512 * (q + 1)], in0=ps[:],
                                                scalar1=b2_sb[:, m:m + 1], scalar2=None, op0=ADD)
                    for t in range(4 * q, 4 * q + 4):
                        tb = FP.tile([128, 2, 128], BF16, tag="ytr", name="yt2", bufs=2)
                        nc.tensor.transpose(tb[:, 0, :], yT[:, 0, 128 * t:128 * (t + 1)], idb_sb[:])
                        nc.tensor.transpose(tb[:, 1, :], yT[:, 1, 128 * t:128 * (t + 1)], idb_sb[:])
                        ot = W4.tile([128, D], F32, tag="ot", name="ot")
                        nc.vector.tensor_tensor(out=ot[:], in0=tb.rearrange("p a b -> p (a b)"),
                                                in1=xtok[:, t, :], op=ADD)
                        nc.sync.dma_start(out=out[128 * t:128 * (t + 1), :], in_=ot[:])

    nc.compile()
    return nc


def _get_compiled():
    global _compiled
    if _compiled is None:
        _compiled = _build()
    return _compiled


def _bva(bv_f):
    a = np.ones((1, H * (HD + 1)), np.float32)
    for h in range(H):
        a[0, 65 * h:65 * h + 64] = bv_f[64 * h:64 * (h + 1)]
    return a.astype(BF)


def _prep_in_maps(tokens, importance, norm1_w, norm1_b, Wq, bq, Wk, bk, Wv, bv,
                  Wo, bo, norm2_w, norm2_b, W1, b1, W2, b2):
    f32 = np.float32
    tokens = np.asarray(tokens, f32)
    importance = np.asarray(importance, f32)

    # fold LN affine params into the following projection weights
    Wq_f = (np.asarray(norm1_w, f32)[:, None] * np.asarray(Wq, f32))
    Wk_f = (np.asarray(norm1_w, f32)[:, None] * np.asarray(Wk, f32))
    Wv_f = (np.asarray(norm1_w, f32)[:, None] * np.asarray(Wv, f32))
    bq_f = np.asarray(norm1_b, f32) @ np.asarray(Wq, f32) + np.asarray(bq, f32)
    bk_f = np.asarray(norm1_b, f32) @ np.asarray(Wk, f32) + np.asarray(bk, f32)
    bv_f = np.asarray(norm1_b, f32) @ np.asarray(Wv, f32) + np.asarray(bv, f32)
    W1_f = (np.asarray(norm2_w, f32)[:, None] * np.asarray(W1, f32))
    b1_f = np.asarray(norm2_b, f32) @ np.asarray(W1, f32) + np.asarray(b1, f32)

    common = {
        "wq": Wq_f.astype(BF), "wk": Wk_f.astype(BF), "wv": Wv_f.astype(BF),
        "wo": np.asarray(Wo, f32).astype(BF),
        "w1": W1_f.astype(BF), "w2": np.asarray(W2, f32).astype(BF),
        "bq": np.ascontiguousarray(bq_f.reshape(2, 128).T.astype(f32)),
        "bk": np.ascontiguousarray(bk_f.reshape(2, 128).T.astype(f32)),
        "bva": _bva(bv_f),
        "bo": np.ascontiguousarray(np.asarray(bo, f32).reshape(2, 128).T),
        "b1": np.ascontiguousarray(b1_f.reshape(4, 128).T.astype(f32)),
        "b2": np.ascontiguousarray(np.asarray(b2, f32).reshape(2, 128).T),
        "idb": np.eye(128, dtype=f32).astype(BF),
    }

    in_maps = []
    for c in range(8):
        b = c // 2
        qh = c % 2
        qs = qh * NQ
        perm = np.r_[qs:qs + NQ, (0 if qh else NQ):(NQ if qh else N)]
        toks = np.ascontiguousarray(tokens[b][perm])
        impp = np.ascontiguousarray(importance[b][perm].reshape(KC, 128).T.astype(f32))
        in_maps.append({"tokens": toks, "imp": impp, **common})
    return in_maps


def _run(in_maps, trace=False):
    nc = _get_compiled()
    return run_bass_kernel_spmd(nc, in_maps, core_ids=list(range(8)), trace=trace)


def _assemble(res):
    out = np.empty((B, N, D), np.float32)
    for c in range(8):
        b = c // 2
        qs = (c % 2) * NQ
        out[b, qs:qs + NQ] = res.results[c]["out"]
    return out


def kernel(**inputs) -> np.ndarray:
    res = _run(_prep_in_maps(**inputs), trace=False)
    return _assemble(res)


def kernel_traced(**inputs):
    """Like kernel() but with NTFF profiling; returns (out, exec_time_ns)."""
    res = _run(_prep_in_maps(**inputs), trace=True)
    return _assemble(res), res.exec_time_ns


# revision 22
# speedup vs baseline: 1.0165x; 1.0059x over previous
"""Trainium2 Bass kernel for nn_AGSISpaBlock (pre-norm MHA + GELU FFN block).

Sharding: 8 cores; core c handles batch b = c//2 and query-half qh = c%2.
Each core receives its batch's tokens PERMUTED so its 2048 local query rows
come first (attention is permutation-invariant over keys, so one SPMD graph
serves all cores). No collectives needed.

Dataflow on each core (all matmuls bf16 with fp32 PSUM accumulation):
  head:  LN1 (token-major, bn_stats, per-region batched sqrt) -> xn^T via PE
         transposes; Q^T/K^T projections; V token-major with importance
         folded in as exp(importance) row scaling (V'' = eimp * [V | 1]).
         LN1 and projections interleaved per 512-token region.
  attn:  keys-major scores S^T[k, q] via head-pair row-packed matmuls
         (heads 2p/2p+1 on PE rows 0-63/64-127 concurrently); exp(S/8) on
         ScalarE straight out of PSUM in 3/2-bank alternating groups;
         ctx^T[hd+1, q] accumulated over key chunks (ones column of V''
         carries the softmax denominator L).  qb-outer / pair-inner loop so
         each 512-query block finishes attention early.
  tail:  per qb: transpose ctx to token-major (L becomes a per-partition
         column) -> 1/L normalize -> transpose back -> Wo -> residual ->
         LN2 -> FFN (exact-erf Gelu) -> final residual -> DMA out.  All
         tail PSUM work goes through one reserved bank so it overlaps the
         attention of later query blocks.
"""

import sys

if "/opt/trn_rl_repo" not in sys.path:
    sys.path.insert(0, "/opt/trn_rl_repo")

import numpy as np
import ml_dtypes

import concourse.bass as bass
import concourse.tile as tile
from concourse import bacc, mybir
from concourse.bass_utils import run_bass_kernel_spmd

F32 = mybir.dt.float32
BF16 = mybir.dt.bfloat16
BF = ml_dtypes.bfloat16

B, N, D = 4, 4096, 256
H, HD = 4, 64
FF = 512
EPS = 1e-5
NQ = N // 2          # local queries per core
KC = N // 128        # key chunks (32)
QB = NQ // 512       # 512-wide query blocks (4)
QT = NQ // 128       # 128-wide query tiles (16)
NR = N // 512        # 512-token regions (8)

_compiled = None


def _build():
    nc = bacc.Bacc("TRN2", target_bir_lowering=False, debug=False, num_devices=8)

    tok = nc.declare_dram_parameter("tokens", [N, D], F32, isOutput=False)
    imp = nc.declare_dram_parameter("imp", [128, KC], F32, isOutput=False)
    wq = nc.declare_dram_parameter("wq", [D, D], BF16, isOutput=False)
    wk = nc.declare_dram_parameter("wk", [D, D], BF16, isOutput=False)
    wv = nc.declare_dram_parameter("wv", [D, D], BF16, isOutput=False)
    wo = nc.declare_dram_parameter("wo", [D, D], BF16, isOutput=False)
    w1 = nc.declare_dram_parameter("w1", [D, FF], BF16, isOutput=False)
    w2 = nc.declare_dram_parameter("w2", [FF, D], BF16, isOutput=False)
    bq = nc.declare_dram_parameter("bq", [128, 2], F32, isOutput=False)
    bk = nc.declare_dram_parameter("bk", [128, 2], F32, isOutput=False)
    bva = nc.declare_dram_parameter("bva", [1, H * 65], BF16, isOutput=False)
    bo = nc.declare_dram_parameter("bo", [128, 2], F32, isOutput=False)
    b1 = nc.declare_dram_parameter("b1", [128, 4], F32, isOutput=False)
    b2 = nc.declare_dram_parameter("b2", [128, 2], F32, isOutput=False)
    idb = nc.declare_dram_parameter("idb", [128, 128], BF16, isOutput=False)
    out = nc.declare_dram_parameter("out", [NQ, D], F32, isOutput=True)

    EXP = mybir.ActivationFunctionType.Exp
    GELU = mybir.ActivationFunctionType.Gelu
    SQRT = mybir.ActivationFunctionType.Sqrt
    SUB = mybir.AluOpType.subtract
    MUL = mybir.AluOpType.mult
    ADD = mybir.AluOpType.add

    with tile.TileContext(nc) as tc:
        with (
            tc.tile_pool(name="singles", bufs=1) as S,
            tc.tile_pool(name="work", bufs=4) as W4,
            tc.tile_pool(name="stats", bufs=4) as ST,
        ):
            # ---- persistent SBUF tensors (chunk-pair merged) ----
            xnT = S.tile([128, 2, N], BF16, tag="xnT", name="xnT")
            qT = S.tile([128, 2, NQ], BF16, tag="qT", name="qT")
            kT = S.tile([128, 2, N], BF16, tag="kT", name="kT")
            v2 = S.tile([128, KC, H * (HD + 1)], BF16, tag="v2", name="v2")
            ctx_sb = S.tile([65, H, NQ], BF16, tag="ctx", name="ctx")
            ctxnT = S.tile([128, 2, NQ], BF16, tag="ctxnT", name="ctxnT")
            aoT = S.tile([128, 2, NQ], BF16, tag="aoT", name="aoT")
            xtok = S.tile([128, QT, D], F32, tag="xtok", name="xtok")
            tokl = S.tile([128, QT, D], F32, tag="tokl", name="tokl")
            xn2T = S.tile([128, 2, NQ], BF16, tag="xn2T", name="xn2T")
            hT = S.tile([128, 4, NQ], BF16, tag="hT", name="hT")
            yT = S.tile([128, 2, NQ], BF16, tag="yT", name="yT")
            mv1 = S.tile([128, N // 128, 2], F32, tag="mv1", name="mv1")
            rs1 = S.tile([128, N // 128], F32, tag="rs1", name="rs1")
            mv2 = S.tile([128, QT, 2], F32, tag="mv2", name="mv2")
            rs2 = S.tile([128, QT], F32, tag="rs2", name="rs2")

            # ---- weights / consts ----
            wq_sb = S.tile([128, 2, D], BF16, tag="wq", name="wq_sb")
            wk_sb = S.tile([128, 2, D], BF16, tag="wk", name="wk_sb")
            wv_sb = S.tile([128, 2, D], BF16, tag="wv", name="wv_sb")
            wo_sb = S.tile([128, 2, D], BF16, tag="wo", name="wo_sb")
            w1_sb = S.tile([128, 2, FF], BF16, tag="w1", name="w1_sb")
            w2_sb = S.tile([128, 4, D], BF16, tag="w2", name="w2_sb")
            for w_sb, w_d in [(wq_sb, wq), (wk_sb, wk), (wv_sb, wv), (wo_sb, wo),
                              (w1_sb, w1), (w2_sb, w2)]:
                nc.sync.dma_start(out=w_sb[:], in_=w_d.rearrange("(c p) d -> p c d", p=128))
            bq_sb = S.tile([128, 2], F32, tag="bq", name="bq_sb")
            bk_sb = S.tile([128, 2], F32, tag="bk", name="bk_sb")
            bva_sb = S.tile([1, H * 65], BF16, tag="bva", name="bva_sb")
            ones1_sb = S.tile([1, 128], BF16, tag="ones1", name="ones1_sb")
            bo_sb = S.tile([128, 2], F32, tag="bo", name="bo_sb")
            b1_sb = S.tile([128, 4], F32, tag="b1", name="b1_sb")
            b2_sb = S.tile([128, 2], F32, tag="b2", name="b2_sb")
            nc.sync.dma_start(out=bq_sb[:], in_=bq[:])
            nc.sync.dma_start(out=bk_sb[:], in_=bk[:])
            nc.sync.dma_start(out=bva_sb[:], in_=bva[:])
            nc.vector.memset(ones1_sb[:], 1.0)
            nc.sync.dma_start(out=bo_sb[:], in_=bo[:])
            nc.sync.dma_start(out=b1_sb[:], in_=b1[:])
            nc.sync.dma_start(out=b2_sb[:], in_=b2[:])
            idb_sb = S.tile([128, 128], BF16, tag="idb", name="idb_sb")
            nc.sync.dma_start(out=idb_sb[:], in_=idb[:])
            imp_sb = S.tile([128, KC], F32, tag="imp", name="imp_sb")
            nc.sync.dma_start(out=imp_sb[:], in_=imp[:])
            eimp_sb = S.tile([128, KC], F32, tag="eimp", name="eimp_sb")
            nc.scalar.activation(out=eimp_sb[:], in_=imp_sb[:], func=EXP)
            eps_sb = S.tile([128, 1], F32, tag="eps", name="eps_sb")
            nc.vector.memset(eps_sb[:], EPS)

            XOR = mybir.AluOpType.bitwise_xor
            SHR = mybir.AluOpType.logical_shift_right
            I32 = mybir.dt.int32

            def quake_rsqrt(var_in, rs_out, n):
                """rs_out[:, :n] = 1/sqrt(var_in + EPS) via DVE-only bit trick."""
                vpe = ST.tile([128, 4], F32, tag="vpe", name="vpe")
                nc.vector.tensor_scalar(out=vpe[:, 0:n], in0=var_in, scalar1=EPS,
                                        scalar2=None, op0=ADD)
                yb = ST.tile([128, 4], I32, tag="yb", name="yb")
                nc.vector.tensor_scalar(out=yb[:, 0:n], in0=vpe[:, 0:n].bitcast(I32),
                                        scalar1=1, scalar2=None, op0=SHR)
                nc.vector.tensor_scalar(out=yb[:, 0:n], in0=yb[:, 0:n], scalar1=-1,
                                        scalar2=None, op0=XOR)
                nc.vector.tensor_scalar(out=yb[:, 0:n], in0=yb[:, 0:n], scalar1=0x5f3759e0,
                                        scalar2=None, op0=ADD)
                y0 = yb[:, 0:n].bitcast(F32)
                t1 = ST.tile([128, 4], F32, tag="t1q", name="t1q")
                y1 = ST.tile([128, 4], F32, tag="y1q", name="y1q")
                nc.vector.tensor_tensor(out=t1[:, 0:n], in0=y0, in1=y0, op=MUL)
                nc.vector.tensor_tensor(out=t1[:, 0:n], in0=t1[:, 0:n], in1=vpe[:, 0:n], op=MUL)
                nc.vector.tensor_scalar(out=t1[:, 0:n], in0=t1[:, 0:n], scalar1=-0.5,
                                        scalar2=1.5, op0=MUL, op1=ADD)
                nc.vector.tensor_tensor(out=y1[:, 0:n], in0=y0, in1=t1[:, 0:n], op=MUL)
                nc.vector.tensor_tensor(out=t1[:, 0:n], in0=y1[:, 0:n], in1=y1[:, 0:n], op=MUL)
                nc.vector.tensor_tensor(out=t1[:, 0:n], in0=t1[:, 0:n], in1=vpe[:, 0:n], op=MUL)
                nc.vector.tensor_scalar(out=t1[:, 0:n], in0=t1[:, 0:n], scalar1=-0.5,
                                        scalar2=1.5, op0=MUL, op1=ADD)
                nc.vector.tensor_tensor(out=rs_out, in0=y1[:, 0:n], in1=t1[:, 0:n], op=MUL)

            # ========= HEAD: LN1 + projections, interleaved per 512-token region =========
            with tc.tile_pool(name="headps", bufs=1, space="PSUM") as HP:
                for r in range(NR):
                    if r < QB:
                        treg = tokl[:, 4 * r:4 * r + 4, :]
                    else:
                        treg = W4.tile([128, 4, D], F32, tag="tokr", name="tokr", bufs=2)[:]
                    nc.sync.dma_start(out=treg,
                                      in_=tok[512 * r:512 * (r + 1), :].rearrange(
                                          "(j p) d -> p j d", p=128))
                    tts = []
                    for j in range(4):
                        i = 4 * r + j
                        tt = treg[:, j, :]
                        tts.append(tt)
                        st = ST.tile([128, 6], F32, tag="st", name="st")
                        nc.vector.bn_stats(out=st[:], in_=tt)
                        nc.vector.bn_aggr(out=mv1[:, i, :], in_=st[:])
                    sd1 = ST.tile([128, 4], F32, tag="sd1", name="sd1")
                    nc.scalar.activation(out=sd1[:], in_=mv1[:, 4 * r:4 * r + 4, 1],
                                         func=SQRT, bias=eps_sb[:], scale=1.0)
                    nc.vector.reciprocal(out=rs1[:, 4 * r:4 * r + 4], in_=sd1[:])
                    for j in range(4):
                        i = 4 * r + j
                        xb = W4.tile([128, D], BF16, tag="xnb", name="xnb")
                        nc.vector.tensor_scalar(out=xb[:], in0=tts[j],
                                                scalar1=mv1[:, i, 0:1],
                                                scalar2=rs1[:, i:i + 1], op0=SUB, op1=MUL)
                        tp = HP.tile([128, 2, 128], BF16, tag="p1t", name="p1t", bufs=2)
                        nc.tensor.transpose(tp[:, 0, :], xb[:, 0:128], idb_sb[:])
                        nc.tensor.transpose(tp[:, 1, :], xb[:, 128:256], idb_sb[:])
                        nc.scalar.copy(out=xnT[:, :, 128 * i:128 * (i + 1)], in_=tp[:])
                    # K projection for this region (bias add on ScalarE)
                    for m in range(2):
                        ps = HP.tile([128, 512], F32, tag="qk", name="kps", bufs=4)
                        for c in range(2):
                            nc.tensor.matmul(ps[:], wk_sb[:, c, 128 * m:128 * (m + 1)],
                                             xnT[:, c, 512 * r:512 * (r + 1)],
                                             start=(c == 0), stop=(c == 1))
                        nc.scalar.add(out=kT[:, m, 512 * r:512 * (r + 1)], in_=ps[:],
                                      add=bk_sb[:, m:m + 1])
                    # Q projection (local queries only)
                    if r < QB:
                        for m in range(2):
                            ps = HP.tile([128, 512], F32, tag="qk", name="qps", bufs=4)
                            for c in range(2):
                                nc.tensor.matmul(ps[:], wq_sb[:, c, 128 * m:128 * (m + 1)],
                                                 xnT[:, c, 512 * r:512 * (r + 1)],
                                                 start=(c == 0), stop=(c == 1))
                            nc.scalar.add(out=qT[:, m, 512 * r:512 * (r + 1)], in_=ps[:],
                                          add=bq_sb[:, m:m + 1])
                    # V'' for this region's key chunks
                    for kc in range(4 * r, 4 * r + 4):
                        ps = HP.tile([128, H * 65], F32, tag="v", name="vps", bufs=2)
                        psr = ps[:].rearrange("p (h j) -> p h j", h=H)
                        for c in range(2):
                            nc.tensor.matmul(psr[:, :, 0:64], xnT[:, c, 128 * kc:128 * (kc + 1)],
                                             wv_sb[:, c, :], start=(c == 0), stop=False,
                                             skip_group_check=True)
                        nc.tensor.matmul(ps[:], ones1_sb[:], bva_sb[:],
                                         start=False, stop=True, skip_group_check=True)
                        nc.vector.tensor_scalar(out=v2[:, kc, :], in0=ps[:],
                                                scalar1=eimp_sb[:, kc:kc + 1], scalar2=None, op0=MUL)

            # ============== ATTENTION + per-qb TAIL (interleaved) ==============
            import collections
            tail_q = collections.deque()

            def drain(k):
                for _ in range(k):
                    if tail_q:
                        tail_q.popleft()()

            with (
                tc.tile_pool(name="p3s", bufs=1, space="PSUM") as P3S,
                tc.tile_pool(name="p3c", bufs=1, space="PSUM") as P3C,
                tc.tile_pool(name="tailps", bufs=1, space="PSUM") as TP,
            ):
                def mk_norm_tr(q, p, t):
                    def f():
                        tp4 = TP.tile([128, 2, 66], BF16, tag="tail", name="tp4")
                        for hp in range(2):
                            nc.tensor.transpose(tp4[:, hp, 0:65],
                                                ctx_sb[0:65, 2 * p + hp, 128 * t:128 * (t + 1)],
                                                idb_sb[0:65, 0:65])
                        rl = ST.tile([128, 2], F32, tag="rl", name="rl")
                        nc.vector.reciprocal(out=rl[:], in_=tp4[:, :, 64:65])
                        ck = W4.tile([128, 128], BF16, tag="ck", name="ck")
                        for hp in range(2):
                            nc.vector.tensor_scalar(out=ck[:, 64 * hp:64 * (hp + 1)],
                                                    in0=tp4[:, hp, 0:64],
                                                    scalar1=rl[:, hp:hp + 1], scalar2=None, op0=MUL)
                        tb = TP.tile([128, 128], BF16, tag="tail", name="tb")
                        nc.tensor.transpose(tb[:], ck[:], idb_sb[:])
                        nc.vector.tensor_copy(out=ctxnT[:, p, 128 * t:128 * (t + 1)], in_=tb[:])
                    return f

                def mk_wo(q, m):
                    def f():
                        ps = TP.tile([128, 512], F32, tag="tail", name="wops")
                        for c in range(2):
                            nc.tensor.matmul(ps[:], wo_sb[:, c, 128 * m:128 * (m + 1)],
                                             ctxnT[:, c, 512 * q:512 * (q + 1)],
                                             start=(c == 0), stop=(c == 1), skip_group_check=True)
                        nc.vector.tensor_scalar(out=aoT[:, m, 512 * q:512 * (q + 1)], in0=ps[:],
                                                scalar1=bo_sb[:, m:m + 1], scalar2=None, op0=ADD)
                    return f

                def mk_resid(q, t):
                    def f():
                        tb = TP.tile([128, 2, 128], BF16, tag="tail", name="aot")
                        nc.tensor.transpose(tb[:, 0, :], aoT[:, 0, 128 * t:128 * (t + 1)], idb_sb[:])
                        nc.tensor.transpose(tb[:, 1, :], aoT[:, 1, 128 * t:128 * (t + 1)], idb_sb[:])
                        nc.vector.tensor_tensor(out=xtok[:, t, :], in0=tb.rearrange("p a b -> p (a b)"),
                                                in1=tokl[:, t, :], op=ADD)
                        st = ST.tile([128, 6], F32, tag="st", name="st")
                        nc.vector.bn_stats(out=st[:], in_=xtok[:, t, :])
                        nc.vector.bn_aggr(out=mv2[:, t, :], in_=st[:])
                    return f

                def mk_rstd2(q):
                    def f():
                        quake_rsqrt(mv2[:, 4 * q:4 * q + 4, 1], rs2[:, 4 * q:4 * q + 4], 4)
                    return f

                def mk_ln2(q, t):
                    def f():
                        xb = W4.tile([128, D], BF16, tag="xnb", name="xnb")
                        nc.vector.tensor_scalar(out=xb[:], in0=xtok[:, t, :], scalar1=mv2[:, t, 0:1],
                                                scalar2=rs2[:, t:t + 1], op0=SUB, op1=MUL)
                        tp2 = TP.tile([128, 2, 128], BF16, tag="tail", name="p8t")
                        nc.tensor.transpose(tp2[:, 0, :], xb[:, 0:128], idb_sb[:])
                        nc.tensor.transpose(tp2[:, 1, :], xb[:, 128:256], idb_sb[:])
                        nc.vector.tensor_copy(out=xn2T[:, :, 128 * t:128 * (t + 1)], in_=tp2[:])
                    return f


                def mk_ffn2(q, m):
                    def f():
                        ps = TP.tile([128, 512], F32, tag="tail", name="yps")
                        for c in range(4):
                            nc.tensor.matmul(ps[:], w2_sb[:, c, 128 * m:128 * (m + 1)],
                                             hT[:, c, 512 * q:512 * (q + 1)],
                                             start=(c == 0), stop=(c == 3), skip_group_check=True)
                        nc.vector.tensor_scalar(out=yT[:, m, 512 * q:512 * (q + 1)], in0=ps[:],
                                                scalar1=b2_sb[:, m:m + 1], scalar2=None, op0=ADD)
                    return f

                def mk_out(q, t):
                    def f():
                        tb = TP.tile([128, 2, 128], BF16, tag="tail", name="yt")
                        nc.tensor.transpose(tb[:, 0, :], yT[:, 0, 128 * t:128 * (t + 1)], idb_sb[:])
                        nc.tensor.transpose(tb[:, 1, :], yT[:, 1, 128 * t:128 * (t + 1)], idb_sb[:])
                        ot = W4.tile([128, D], F32, tag="ot", name="ot")
                        nc.vector.tensor_tensor(out=ot[:], in0=tb.rearrange("p a b -> p (a b)"),
                                                in1=xtok[:, t, :], op=ADD)
                        nc.sync.dma_start(out=out[128 * t:128 * (t + 1), :], in_=ot[:])
                    return f

                for q in range(QB):
                    for p in range(2):
                        cps = P3C.tile([65, 2, 512], F32, tag="ctxps", name="ctxps")
                        slots = [(kc, hp) for kc in range(KC) for hp in range(2)]
                        sizes = [3, 2] * 12 + [3, 1]
                        g = 0
                        pending = []

                        def emit_ctx(pend):
                            pt_, slots_ = pend
                            for j_, (kc_, hp_) in enumerate(slots_):
                                h_ = 2 * p + hp_
                                nc.tensor.matmul(cps[:, hp_, :], v2[:, kc_, 65 * h_:65 * (h_ + 1)],
                                                 pt_[:, j_, :], start=(kc_ == 0),
                                                 stop=(kc_ == KC - 1), skip_group_check=True)

                        for n in sizes:
                            tagn = "sgA" if n == 3 else "sgB"
                            sg = P3S.tile([128, 3 if n == 3 else 2, 512], F32,
                                          tag=tagn, name=tagn)
                            for j in range(n):
                                kc, hp = slots[g + j]
                                nc.tensor.matmul(
                                    sg[:, j, :],
                                    kT[64 * hp:64 * (hp + 1), p, 128 * kc:128 * (kc + 1)],
                                    qT[64 * hp:64 * (hp + 1), p, 512 * q:512 * (q + 1)],
                                    start=True, stop=True, skip_group_check=True)
                            pt = W4.tile([128, 3 if n == 3 else 2, 512], BF16,
                                         tag=f"pt{tagn}", name=f"pt{tagn}", bufs=4)
                            nc.scalar.activation(out=pt[:, 0:n, :], in_=sg[:, 0:n, :],
                                                 func=EXP, scale=0.125)
                            pending.append((pt, [slots[g + j] for j in range(n)]))
                            if len(pending) > 4:
                                emit_ctx(pending.pop(0))
                            g += n
                            drain(1)
                        for pend in pending:
                            emit_ctx(pend)
                        for hp in range(2):
                            nc.vector.tensor_copy(out=ctx_sb[:, 2 * p + hp, 512 * q:512 * (q + 1)],
                                                  in_=cps[:, hp, :])
                        for t in range(4 * q, 4 * q + 4):
                            tail_q.append(mk_norm_tr(q, p, t))
                    for m in range(2):
                        tail_q.append(mk_wo(q, m))
                    for t in range(4 * q, 4 * q + 4):
                        tail_q.append(mk_resid(q, t))
                    tail_q.append(mk_rstd2(q))
                    for t in range(4 * q, 4 * q + 4):
                        tail_q.append(mk_ln2(q, t))
                drain(len(tail_q))

            # ============== gelu + FFN2 + output (post-attention) ==============
            with tc.tile_pool(name="ffps", bufs=1, space="PSUM") as FP:
                for q in range(QB):
                    for f_ in range(4):
                        ps = FP.tile([128, 512], F32, tag="ff", name="ffps2", bufs=4)
                        for c in range(2):
                            nc.tensor.matmul(ps[:], w1_sb[:, c, 128 * f_:128 * (f_ + 1)],
                                             xn2T[:, c, 512 * q:512 * (q + 1)],
                                             start=(c == 0), stop=(c == 1), skip_group_check=True)
                        nc.scalar.activation(out=hT[:, f_, 512 * q:512 * (q + 1)], in_=ps[:],
                                             func=GELU, bias=b1_sb[:, f_:f_ + 1], scale=1.0)
                    for m in range(2):
                        ps = FP.tile([128, 512], F32, tag="y2", name="yps2", bufs=2)
                        for c in range(4):
                            nc.tensor.matmul(ps[:], w2_sb[:, c, 128 * m:128 * (m + 1)],
                                             hT[:, c, 512 * q:512 * (q + 1)],
                                             start=(c == 0), stop=(c == 3), skip_group_check=True)
                        nc.vector.tensor_scalar(out=yT[:, m, 512 * q:512 * (q + 1)], in0=ps[:],
                                                scalar1=b2_sb[:, m:m + 1], scalar2=None, op0=ADD)
                    for t in range(4 * q, 4 * q + 4):
                        tb = FP.tile([128, 2, 128], BF16, tag="ytr", name="yt2", bufs=2)
                        nc.tensor.transpose(tb[:, 0, :], yT[:, 0, 128 * t:128 * (t + 1)], idb_sb[:])
                        nc.tensor.transpose(tb[:, 1, :], yT[:, 1, 128 * t:128 * (t + 1)], idb_sb[:])
                        ot = W4.tile([128, D], F32, tag="ot", name="ot")
                        nc.vector.tensor_tensor(out=ot[:], in0=tb.rearrange("p a b -> p (a b)"),
                                                in1=xtok[:, t, :], op=ADD)
                        nc.sync.dma_start(out=out[128 * t:128 * (t + 1), :], in_=ot[:])

    nc.compile()
    return nc


def _get_compiled():
    global _compiled
    if _compiled is None:
        _compiled = _build()
    return _compiled


def _bva(bv_f):
    a = np.ones((1, H * (HD + 1)), np.float32)
    for h in range(H):
        a[0, 65 * h:65 * h + 64] = bv_f[64 * h:64 * (h + 1)]
    return a.astype(BF)


def _prep_in_maps(tokens, importance, norm1_w, norm1_b, Wq, bq, Wk, bk, Wv, bv,
                  Wo, bo, norm2_w, norm2_b, W1, b1, W2, b2):
    f32 = np.float32
    tokens = np.asarray(tokens, f32)
    importance = np.asarray(importance, f32)

    # fold LN affine params into the following projection weights
    Wq_f = (np.asarray(norm1_w, f32)[:, None] * np.asarray(Wq, f32))
    Wk_f = (np.asarray(norm1_w, f32)[:, None] * np.asarray(Wk, f32))
    Wv_f = (np.asarray(norm1_w, f32)[:, None] * np.asarray(Wv, f32))
    bq_f = np.asarray(norm1_b, f32) @ np.asarray(Wq, f32) + np.asarray(bq, f32)
    bk_f = np.asarray(norm1_b, f32) @ np.asarray(Wk, f32) + np.asarray(bk, f32)
    bv_f = np.asarray(norm1_b, f32) @ np.asarray(Wv, f32) + np.asarray(bv, f32)
    W1_f = (np.asarray(norm2_w, f32)[:, None] * np.asarray(W1, f32))
    b1_f = np.asarray(norm2_b, f32) @ np.asarray(W1, f32) + np.asarray(b1, f32)

    common = {
        "wq": Wq_f.astype(BF), "wk": Wk_f.astype(BF), "wv": Wv_f.astype(BF),
        "wo": np.asarray(Wo, f32).astype(BF),
        "w1": W1_f.astype(BF), "w2": np.asarray(W2, f32).astype(BF),
        "bq": np.ascontiguousarray(bq_f.reshape(2, 128).T.astype(f32)),
        "bk": np.ascontiguousarray(bk_f.reshape(2, 128).T.astype(f32)),
        "bva": _bva(bv_f),
        "bo": np.ascontiguousarray(np.asarray(bo, f32).reshape(2, 128).T),
        "b1": np.ascontiguousarray(b1_f.reshape(4, 128).T.astype(f32)),
        "b2": np.ascontiguousarray(np.asarray(b2, f32).reshape(2, 128).T),
        "idb": np.eye(128, dtype=f32).astype(BF),
    }

    in_maps = []
    for c in range(8):
        b = c // 2
        qh = c % 2
        qs = qh * NQ
        perm = np.r_[qs:qs + NQ, (0 if qh else NQ):(NQ if qh else N)]
        toks = np.ascontiguousarray(tokens[b][perm])
        impp = np.ascontiguousarray(importance[b][perm].reshape(KC, 128).T.astype(f32))
        in_maps.append({"tokens": toks, "imp": impp, **common})
    return in_maps


def _run(in_maps, trace=False):
    nc = _get_compiled()
    return run_bass_kernel_spmd(nc, in_maps, core_ids=list(range(8)), trace=trace)


def _assemble(res):
    out = np.empty((B, N, D), np.float32)
    for c in range(8):
        b = c // 2
        qs = (c % 2) * NQ
        out[b, qs:qs + NQ] = res.results[c]["out"]
    return out


def kernel(**inputs) -> np.ndarray:
    res = _run(_prep_in_maps(**inputs), trace=False)
    return _assemble(res)


def kernel_traced(**inputs):
    """Like kernel() but with NTFF profiling; returns (out, exec_time_ns)."""
    res = _run(_prep_in_maps(**inputs), trace=True)
    return _assemble(res), res.exec_time_ns


# revision 23
# speedup vs baseline: 1.0215x; 1.0049x over previous
"""Trainium2 Bass kernel for nn_AGSISpaBlock (pre-norm MHA + GELU FFN block).

Sharding: 8 cores; core c handles batch b = c//2 and query-half qh = c%2.
Each core receives its batch's tokens PERMUTED so its 2048 local query rows
come first (attention is permutation-invariant over keys, so one SPMD graph
serves all cores). No collectives needed.

Dataflow on each core (all matmuls bf16 with fp32 PSUM accumulation):
  head:  LN1 (token-major, bn_stats, per-region batched sqrt) -> xn^T via PE
         transposes; Q^T/K^T projections; V token-major with importance
         folded in as exp(importance) row scaling (V'' = eimp * [V | 1]).
         LN1 and projections interleaved per 512-token region.
  attn:  keys-major scores S^T[k, q] via head-pair row-packed matmuls
         (heads 2p/2p+1 on PE rows 0-63/64-127 concurrently); exp(S/8) on
         ScalarE straight out of PSUM in 3/2-bank alternating groups;
         ctx^T[hd+1, q] accumulated over key chunks (ones column of V''
         carries the softmax denominator L).  qb-outer / pair-inner loop so
         each 512-query block finishes attention early.
  tail:  per qb: transpose ctx to token-major (L becomes a per-partition
         column) -> 1/L normalize -> transpose back -> Wo -> residual ->
         LN2 -> FFN (exact-erf Gelu) -> final residual -> DMA out.  All
         tail PSUM work goes through one reserved bank so it overlaps the
         attention of later query blocks.
"""

import sys

if "/opt/trn_rl_repo" not in sys.path:
    sys.path.insert(0, "/opt/trn_rl_repo")

import numpy as np
import ml_dtypes

import concourse.bass as bass
import concourse.tile as tile
from concourse import bacc, mybir
from concourse.bass_utils import run_bass_kernel_spmd

F32 = mybir.dt.float32
BF16 = mybir.dt.bfloat16
BF = ml_dtypes.bfloat16

B, N, D = 4, 4096, 256
H, HD = 4, 64
FF = 512
EPS = 1e-5
NQ = N // 2          # local queries per core
KC = N // 128        # key chunks (32)
QB = NQ // 512       # 512-wide query blocks (4)
QT = NQ // 128       # 128-wide query tiles (16)
NR = N // 512        # 512-token regions (8)

_compiled = None


def _build():
    nc = bacc.Bacc("TRN2", target_bir_lowering=False, debug=False, num_devices=8)

    tok = nc.declare_dram_parameter("tokens", [N, D], F32, isOutput=False)
    imp = nc.declare_dram_parameter("imp", [128, KC], F32, isOutput=False)
    wq = nc.declare_dram_parameter("wq", [D, D], BF16, isOutput=False)
    wk = nc.declare_dram_parameter("wk", [D, D], BF16, isOutput=False)
    wv = nc.declare_dram_parameter("wv", [D, D], BF16, isOutput=False)
    wo = nc.declare_dram_parameter("wo", [D, D], BF16, isOutput=False)
    w1 = nc.declare_dram_parameter("w1", [D, FF], BF16, isOutput=False)
    w2 = nc.declare_dram_parameter("w2", [FF, D], BF16, isOutput=False)
    bq = nc.declare_dram_parameter("bq", [128, 2], F32, isOutput=False)
    bk = nc.declare_dram_parameter("bk", [128, 2], F32, isOutput=False)
    bva = nc.declare_dram_parameter("bva", [1, H * 65], BF16, isOutput=False)
    bo = nc.declare_dram_parameter("bo", [128, 2], F32, isOutput=False)
    b1 = nc.declare_dram_parameter("b1", [128, 4], F32, isOutput=False)
    b2 = nc.declare_dram_parameter("b2", [128, 2], F32, isOutput=False)
    idb = nc.declare_dram_parameter("idb", [128, 128], BF16, isOutput=False)
    out = nc.declare_dram_parameter("out", [NQ, D], F32, isOutput=True)

    EXP = mybir.ActivationFunctionType.Exp
    GELU = mybir.ActivationFunctionType.Gelu
    SQRT = mybir.ActivationFunctionType.Sqrt
    SUB = mybir.AluOpType.subtract
    MUL = mybir.AluOpType.mult
    ADD = mybir.AluOpType.add

    with tile.TileContext(nc) as tc:
        with (
            tc.tile_pool(name="singles", bufs=1) as S,
            tc.tile_pool(name="work", bufs=4) as W4,
            tc.tile_pool(name="stats", bufs=4) as ST,
        ):
            # ---- persistent SBUF tensors (chunk-pair merged) ----
            xnT = S.tile([128, 2, N], BF16, tag="xnT", name="xnT")
            qT = S.tile([128, 2, NQ], BF16, tag="qT", name="qT")
            kT = S.tile([128, 2, N], BF16, tag="kT", name="kT")
            v2 = S.tile([128, KC, H * (HD + 1)], BF16, tag="v2", name="v2")
            ctx_sb = S.tile([65, H, NQ], BF16, tag="ctx", name="ctx")
            ctxnT = S.tile([128, 2, NQ], BF16, tag="ctxnT", name="ctxnT")
            aoT = S.tile([128, 2, NQ], BF16, tag="aoT", name="aoT")
            xtok = S.tile([128, QT, D], F32, tag="xtok", name="xtok")
            tokl = S.tile([128, QT, D], F32, tag="tokl", name="tokl")
            xn2T = S.tile([128, 2, NQ], BF16, tag="xn2T", name="xn2T")
            hT = S.tile([128, 4, NQ], BF16, tag="hT", name="hT")
            yT = S.tile([128, 2, NQ], BF16, tag="yT", name="yT")
            mv1 = S.tile([128, N // 128, 2], F32, tag="mv1", name="mv1")
            rs1 = S.tile([128, N // 128], F32, tag="rs1", name="rs1")
            mv2 = S.tile([128, QT, 2], F32, tag="mv2", name="mv2")
            rs2 = S.tile([128, QT], F32, tag="rs2", name="rs2")

            # ---- weights / consts ----
            wq_sb = S.tile([128, 2, D], BF16, tag="wq", name="wq_sb")
            wk_sb = S.tile([128, 2, D], BF16, tag="wk", name="wk_sb")
            wv_sb = S.tile([128, 2, D], BF16, tag="wv", name="wv_sb")
            wo_sb = S.tile([128, 2, D], BF16, tag="wo", name="wo_sb")
            w1_sb = S.tile([128, 2, FF], BF16, tag="w1", name="w1_sb")
            w2_sb = S.tile([128, 4, D], BF16, tag="w2", name="w2_sb")
            for w_sb, w_d in [(wq_sb, wq), (wk_sb, wk), (wv_sb, wv), (wo_sb, wo),
                              (w1_sb, w1), (w2_sb, w2)]:
                nc.sync.dma_start(out=w_sb[:], in_=w_d.rearrange("(c p) d -> p c d", p=128))
            bq_sb = S.tile([128, 2], F32, tag="bq", name="bq_sb")
            bk_sb = S.tile([128, 2], F32, tag="bk", name="bk_sb")
            bva_sb = S.tile([1, H * 65], BF16, tag="bva", name="bva_sb")
            ones1_sb = S.tile([1, 128], BF16, tag="ones1", name="ones1_sb")
            bo_sb = S.tile([128, 2], F32, tag="bo", name="bo_sb")
            b1_sb = S.tile([128, 4], F32, tag="b1", name="b1_sb")
            b2_sb = S.tile([128, 2], F32, tag="b2", name="b2_sb")
            nc.sync.dma_start(out=bq_sb[:], in_=bq[:])
            nc.sync.dma_start(out=bk_sb[:], in_=bk[:])
            nc.sync.dma_start(out=bva_sb[:], in_=bva[:])
            nc.vector.memset(ones1_sb[:], 1.0)
            nc.sync.dma_start(out=bo_sb[:], in_=bo[:])
            nc.sync.dma_start(out=b1_sb[:], in_=b1[:])
            nc.sync.dma_start(out=b2_sb[:], in_=b2[:])
            idb_sb = S.tile([128, 128], BF16, tag="idb", name="idb_sb")
            nc.sync.dma_start(out=idb_sb[:], in_=idb[:])
            imp_sb = S.tile([128, KC], F32, tag="imp", name="imp_sb")
            nc.sync.dma_start(out=imp_sb[:], in_=imp[:])
            eimp_sb = S.tile([128, KC], F32, tag="eimp", name="eimp_sb")
            nc.scalar.activation(out=eimp_sb[:], in_=imp_sb[:], func=EXP)
            eps_sb = S.tile([128, 1], F32, tag="eps", name="eps_sb")
            nc.vector.memset(eps_sb[:], EPS)

            XOR = mybir.AluOpType.bitwise_xor
            SHR = mybir.AluOpType.logical_shift_right
            I32 = mybir.dt.int32

            def quake_rsqrt(var_in, rs_out, n):
                """rs_out[:, :n] = 1/sqrt(var_in + EPS) via DVE-only bit trick."""
                vpe = ST.tile([128, 4], F32, tag="vpe", name="vpe")
                nc.vector.tensor_scalar(out=vpe[:, 0:n], in0=var_in, scalar1=EPS,
                                        scalar2=None, op0=ADD)
                yb = ST.tile([128, 4], I32, tag="yb", name="yb")
                nc.vector.tensor_scalar(out=yb[:, 0:n], in0=vpe[:, 0:n].bitcast(I32),
                                        scalar1=1, scalar2=None, op0=SHR)
                nc.vector.tensor_scalar(out=yb[:, 0:n], in0=yb[:, 0:n], scalar1=-1,
                                        scalar2=None, op0=XOR)
                nc.vector.tensor_scalar(out=yb[:, 0:n], in0=yb[:, 0:n], scalar1=0x5f3759e0,
                                        scalar2=None, op0=ADD)
                y0 = yb[:, 0:n].bitcast(F32)
                t1 = ST.tile([128, 4], F32, tag="t1q", name="t1q")
                y1 = ST.tile([128, 4], F32, tag="y1q", name="y1q")
                nc.vector.tensor_tensor(out=t1[:, 0:n], in0=y0, in1=y0, op=MUL)
                nc.vector.tensor_tensor(out=t1[:, 0:n], in0=t1[:, 0:n], in1=vpe[:, 0:n], op=MUL)
                nc.vector.tensor_scalar(out=t1[:, 0:n], in0=t1[:, 0:n], scalar1=-0.5,
                                        scalar2=1.5, op0=MUL, op1=ADD)
                nc.vector.tensor_tensor(out=y1[:, 0:n], in0=y0, in1=t1[:, 0:n], op=MUL)
                nc.vector.tensor_tensor(out=t1[:, 0:n], in0=y1[:, 0:n], in1=y1[:, 0:n], op=MUL)
                nc.vector.tensor_tensor(out=t1[:, 0:n], in0=t1[:, 0:n], in1=vpe[:, 0:n], op=MUL)
                nc.vector.tensor_scalar(out=t1[:, 0:n], in0=t1[:, 0:n], scalar1=-0.5,
                                        scalar2=1.5, op0=MUL, op1=ADD)
                nc.vector.tensor_tensor(out=rs_out, in0=y1[:, 0:n], in1=t1[:, 0:n], op=MUL)

            # ========= HEAD: LN1 + projections, interleaved per 512-token region =========
            with tc.tile_pool(name="headps", bufs=1, space="PSUM") as HP:
                for r in range(NR):
                    if r < QB:
                        treg = tokl[:, 4 * r:4 * r + 4, :]
                    else:
                        treg = W4.tile([128, 4, D], F32, tag="tokr", name="tokr", bufs=2)[:]
                    nc.sync.dma_start(out=treg,
                                      in_=tok[512 * r:512 * (r + 1), :].rearrange(
                                          "(j p) d -> p j d", p=128))
                    tts = []
                    for j in range(4):
                        i = 4 * r + j
                        tt = treg[:, j, :]
                        tts.append(tt)
                        st = ST.tile([128, 6], F32, tag="st", name="st")
                        nc.vector.bn_stats(out=st[:], in_=tt)
                        nc.vector.bn_aggr(out=mv1[:, i, :], in_=st[:])
                    quake_rsqrt(mv1[:, 4 * r:4 * r + 4, 1], rs1[:, 4 * r:4 * r + 4], 4)
                    for j in range(4):
                        i = 4 * r + j
                        xb = W4.tile([128, D], BF16, tag="xnb", name="xnb")
                        nc.vector.tensor_scalar(out=xb[:], in0=tts[j],
                                                scalar1=mv1[:, i, 0:1],
                                                scalar2=rs1[:, i:i + 1], op0=SUB, op1=MUL)
                        tp = HP.tile([128, 2, 128], BF16, tag="p1t", name="p1t", bufs=2)
                        nc.tensor.transpose(tp[:, 0, :], xb[:, 0:128], idb_sb[:])
                        nc.tensor.transpose(tp[:, 1, :], xb[:, 128:256], idb_sb[:])
                        nc.scalar.copy(out=xnT[:, :, 128 * i:128 * (i + 1)], in_=tp[:])
                    # K projection for this region (bias add on ScalarE)
                    for m in range(2):
                        ps = HP.tile([128, 512], F32, tag="qk", name="kps", bufs=4)
                        for c in range(2):
                            nc.tensor.matmul(ps[:], wk_sb[:, c, 128 * m:128 * (m + 1)],
                                             xnT[:, c, 512 * r:512 * (r + 1)],
                                             start=(c == 0), stop=(c == 1))
                        nc.scalar.add(out=kT[:, m, 512 * r:512 * (r + 1)], in_=ps[:],
                                      add=bk_sb[:, m:m + 1])
                    # Q projection (local queries only)
                    if r < QB:
                        for m in range(2):
                            ps = HP.tile([128, 512], F32, tag="qk", name="qps", bufs=4)
                            for c in range(2):
                                nc.tensor.matmul(ps[:], wq_sb[:, c, 128 * m:128 * (m + 1)],
                                                 xnT[:, c, 512 * r:512 * (r + 1)],
                                                 start=(c == 0), stop=(c == 1))
                            nc.scalar.add(out=qT[:, m, 512 * r:512 * (r + 1)], in_=ps[:],
                                          add=bq_sb[:, m:m + 1])
                    # V'' for this region's key chunks
                    for kc in range(4 * r, 4 * r + 4):
                        ps = HP.tile([128, H * 65], F32, tag="v", name="vps", bufs=2)
                        psr = ps[:].rearrange("p (h j) -> p h j", h=H)
                        for c in range(2):
                            nc.tensor.matmul(psr[:, :, 0:64], xnT[:, c, 128 * kc:128 * (kc + 1)],
                                             wv_sb[:, c, :], start=(c == 0), stop=False,
                                             skip_group_check=True)
                        nc.tensor.matmul(ps[:], ones1_sb[:], bva_sb[:],
                                         start=False, stop=True, skip_group_check=True)
                        nc.vector.tensor_scalar(out=v2[:, kc, :], in0=ps[:],
                                                scalar1=eimp_sb[:, kc:kc + 1], scalar2=None, op0=MUL)

            # ============== ATTENTION + per-qb TAIL (interleaved) ==============
            import collections
            tail_q = collections.deque()

            def drain(k):
                for _ in range(k):
                    if tail_q:
                        tail_q.popleft()()

            with (
                tc.tile_pool(name="p3s", bufs=1, space="PSUM") as P3S,
                tc.tile_pool(name="p3c", bufs=1, space="PSUM") as P3C,
                tc.tile_pool(name="tailps", bufs=1, space="PSUM") as TP,
            ):
                def mk_norm_tr(q, p, t):
                    def f():
                        tp4 = TP.tile([128, 2, 66], BF16, tag="tail", name="tp4")
                        for hp in range(2):
                            nc.tensor.transpose(tp4[:, hp, 0:65],
                                                ctx_sb[0:65, 2 * p + hp, 128 * t:128 * (t + 1)],
                                                idb_sb[0:65, 0:65])
                        rl = ST.tile([128, 2], F32, tag="rl", name="rl")
                        nc.vector.reciprocal(out=rl[:], in_=tp4[:, :, 64:65])
                        ck = W4.tile([128, 128], BF16, tag="ck", name="ck")
                        for hp in range(2):
                            nc.vector.tensor_scalar(out=ck[:, 64 * hp:64 * (hp + 1)],
                                                    in0=tp4[:, hp, 0:64],
                                                    scalar1=rl[:, hp:hp + 1], scalar2=None, op0=MUL)
                        tb = TP.tile([128, 128], BF16, tag="tail", name="tb")
                        nc.tensor.transpose(tb[:], ck[:], idb_sb[:])
                        nc.vector.tensor_copy(out=ctxnT[:, p, 128 * t:128 * (t + 1)], in_=tb[:])
                    return f

                def mk_wo(q, m):
                    def f():
                        ps = TP.tile([128, 512], F32, tag="tail", name="wops")
                        for c in range(2):
                            nc.tensor.matmul(ps[:], wo_sb[:, c, 128 * m:128 * (m + 1)],
                                             ctxnT[:, c, 512 * q:512 * (q + 1)],
                                             start=(c == 0), stop=(c == 1), skip_group_check=True)
                        nc.vector.tensor_scalar(out=aoT[:, m, 512 * q:512 * (q + 1)], in0=ps[:],
                                                scalar1=bo_sb[:, m:m + 1], scalar2=None, op0=ADD)
                    return f

                def mk_resid(q, t):
                    def f():
                        tb = TP.tile([128, 2, 128], BF16, tag="tail", name="aot")
                        nc.tensor.transpose(tb[:, 0, :], aoT[:, 0, 128 * t:128 * (t + 1)], idb_sb[:])
                        nc.tensor.transpose(tb[:, 1, :], aoT[:, 1, 128 * t:128 * (t + 1)], idb_sb[:])
                        nc.vector.tensor_tensor(out=xtok[:, t, :], in0=tb.rearrange("p a b -> p (a b)"),
                                                in1=tokl[:, t, :], op=ADD)
                        st = ST.tile([128, 6], F32, tag="st", name="st")
                        nc.vector.bn_stats(out=st[:], in_=xtok[:, t, :])
                        nc.vector.bn_aggr(out=mv2[:, t, :], in_=st[:])
                    return f

                def mk_rstd2(q):
                    def f():
                        quake_rsqrt(mv2[:, 4 * q:4 * q + 4, 1], rs2[:, 4 * q:4 * q + 4], 4)
                    return f

                def mk_ln2(q, t):
                    def f():
                        xb = W4.tile([128, D], BF16, tag="xnb", name="xnb")
                        nc.vector.tensor_scalar(out=xb[:], in0=xtok[:, t, :], scalar1=mv2[:, t, 0:1],
                                                scalar2=rs2[:, t:t + 1], op0=SUB, op1=MUL)
                        tp2 = TP.tile([128, 2, 128], BF16, tag="tail", name="p8t")
                        nc.tensor.transpose(tp2[:, 0, :], xb[:, 0:128], idb_sb[:])
                        nc.tensor.transpose(tp2[:, 1, :], xb[:, 128:256], idb_sb[:])
                        nc.vector.tensor_copy(out=xn2T[:, :, 128 * t:128 * (t + 1)], in_=tp2[:])
                    return f


                def mk_ffn2(q, m):
                    def f():
                        ps = TP.tile([128, 512], F32, tag="tail", name="yps")
                        for c in range(4):
                            nc.tensor.matmul(ps[:], w2_sb[:, c, 128 * m:128 * (m + 1)],
                                             hT[:, c, 512 * q:512 * (q + 1)],
                                             start=(c == 0), stop=(c == 3), skip_group_check=True)
                        nc.vector.tensor_scalar(out=yT[:, m, 512 * q:512 * (q + 1)], in0=ps[:],
                                                scalar1=b2_sb[:, m:m + 1], scalar2=None, op0=ADD)
                    return f

                def mk_out(q, t):
                    def f():
                        tb = TP.tile([128, 2, 128], BF16, tag="tail", name="yt")
                        nc.tensor.transpose(tb[:, 0, :], yT[:, 0, 128 * t:128 * (t + 1)], idb_sb[:])
                        nc.tensor.transpose(tb[:, 1, :], yT[:, 1, 128 * t:128 * (t + 1)], idb_sb[:])
                        ot = W4.tile([128, D], F32, tag="ot", name="ot")
                        nc.vector.tensor_tensor(out=ot[:], in0=tb.rearrange("p a b -> p (a b)"),
                                                in1=xtok[:, t, :], op=ADD)
                        nc.sync.dma_start(out=out[128 * t:128 * (t + 1), :], in_=ot[:])
                    return f

                for q in range(QB):
                    for p in range(2):
                        cps = P3C.tile([65, 2, 512], F32, tag="ctxps", name="ctxps")
                        slots = [(kc, hp) for kc in range(KC) for hp in range(2)]
                        sizes = [3, 2] * 12 + [3, 1]
                        g = 0
                        pending = []

                        def emit_ctx(pend):
                            pt_, slots_ = pend
                            for j_, (kc_, hp_) in enumerate(slots_):
                                h_ = 2 * p + hp_
                                nc.tensor.matmul(cps[:, hp_, :], v2[:, kc_, 65 * h_:65 * (h_ + 1)],
                                                 pt_[:, j_, :], start=(kc_ == 0),
                                                 stop=(kc_ == KC - 1), skip_group_check=True)

                        for n in sizes:
                            tagn = "sgA" if n == 3 else "sgB"
                            sg = P3S.tile([128, 3 if n == 3 else 2, 512], F32,
                                          tag=tagn, name=tagn)
                            for j in range(n):
                                kc, hp = slots[g + j]
                                nc.tensor.matmul(
                                    sg[:, j, :],
                                    kT[64 * hp:64 * (hp + 1), p, 128 * kc:128 * (kc + 1)],
                                    qT[64 * hp:64 * (hp + 1), p, 512 * q:512 * (q + 1)],
                                    start=True, stop=True, skip_group_check=True)
                            pt = W4.tile([128, 3 if n == 3 else 2, 512], BF16,
                                         tag=f"pt{tagn}", name=f"pt{tagn}", bufs=4)
                            nc.scalar.activation(out=pt[:, 0:n, :], in_=sg[:, 0:n, :],
                                                 func=EXP, scale=0.125)
                            pending.append((pt, [slots[g + j] for j in range(n)]))
                            if len(pending) > 4:
                                emit_ctx(pending.pop(0))
                            g += n
                            drain(1)
                        for pend in pending:
                            emit_ctx(pend)
                        for hp in range(2):
                            nc.vector.tensor_copy(out=ctx_sb[:, 2 * p + hp, 512 * q:512 * (q + 1)],
                                                  in_=cps[:, hp, :])
                        for t in range(4 * q, 4 * q + 4):
                            tail_q.append(mk_norm_tr(q, p, t))
                    for m in range(2):
                        tail_q.append(mk_wo(q, m))
                    for t in range(4 * q, 4 * q + 4):
                        tail_q.append(mk_resid(q, t))
                    tail_q.append(mk_rstd2(q))
                    for t in range(4 * q, 4 * q + 4):
                        tail_q.append(mk_ln2(q, t))
                drain(len(tail_q))

            # ============== gelu + FFN2 + output (post-attention) ==============
            with tc.tile_pool(name="ffps", bufs=1, space="PSUM") as FP:
                for q in range(QB):
                    for f_ in range(4):
                        ps = FP.tile([128, 512], F32, tag="ff", name="ffps2", bufs=4)
                        for c in range(2):
                            nc.tensor.matmul(ps[:], w1_sb[:, c, 128 * f_:128 * (f_ + 1)],
                                             xn2T[:, c, 512 * q:512 * (q + 1)],
                                             start=(c == 0), stop=(c == 1), skip_group_check=True)
                        nc.scalar.activation(out=hT[:, f_, 512 * q:512 * (q + 1)], in_=ps[:],
                                             func=GELU, bias=b1_sb[:, f_:f_ + 1], scale=1.0)
                    for m in range(2):
                        ps = FP.tile([128, 512], F32, tag="y2", name="yps2", bufs=2)
                        for c in range(4):
                            nc.tensor.matmul(ps[:], w2_sb[:, c, 128 * m:128 * (m + 1)],
                                             hT[:, c, 512 * q:512 * (q + 1)],
                                             start=(c == 0), stop=(c == 3), skip_group_check=True)
                        nc.vector.tensor_scalar(out=yT[:, m, 512 * q:512 * (q + 1)], in0=ps[:],
                                                scalar1=b2_sb[:, m:m + 1], scalar2=None, op0=ADD)
                    oq = W4.tile([128, 4, D], F32, tag="oq", name="oq", bufs=2)
                    for j, t in enumerate(range(4 * q, 4 * q + 4)):
                        tb = FP.tile([128, 2, 128], BF16, tag="ytr", name="yt2", bufs=2)
                        nc.tensor.transpose(tb[:, 0, :], yT[:, 0, 128 * t:128 * (t + 1)], idb_sb[:])
                        nc.tensor.transpose(tb[:, 1, :], yT[:, 1, 128 * t:128 * (t + 1)], idb_sb[:])
                        nc.vector.tensor_tensor(out=oq[:, j, :], in0=tb.rearrange("p a b -> p (a b)"),
                                                in1=xtok[:, t, :], op=ADD)
                    nc.sync.dma_start(out=out[512 * q:512 * (q + 1), :].rearrange(
                        "(j p) d -> p j d", p=128), in_=oq[:])

    nc.compile()
    return nc


def _get_compiled():
    global _compiled
    if _compiled is None:
        _compiled = _build()
    return _compiled


def _bva(bv_f):
    a = np.ones((1, H * (HD + 1)), np.float32)
    for h in range(H):
        a[0, 65 * h:65 * h + 64] = bv_f[64 * h:64 * (h + 1)]
    return a.astype(BF)


def _prep_in_maps(tokens, importance, norm1_w, norm1_b, Wq, bq, Wk, bk, Wv, bv,
                  Wo, bo, norm2_w, norm2_b, W1, b1, W2, b2):
    f32 = np.float32
    tokens = np.asarray(tokens, f32)
    importance = np.asarray(importance, f32)

    # fold LN affine params into the following projection weights
    Wq_f = (np.asarray(norm1_w, f32)[:, None] * np.asarray(Wq, f32))
    Wk_f = (np.asarray(norm1_w, f32)[:, None] * np.asarray(Wk, f32))
    Wv_f = (np.asarray(norm1_w, f32)[:, None] * np.asarray(Wv, f32))
    bq_f = np.asarray(norm1_b, f32) @ np.asarray(Wq, f32) + np.asarray(bq, f32)
    bk_f = np.asarray(norm1_b, f32) @ np.asarray(Wk, f32) + np.asarray(bk, f32)
    bv_f = np.asarray(norm1_b, f32) @ np.asarray(Wv, f32) + np.asarray(bv, f32)
    W1_f = (np.asarray(norm2_w, f32)[:, None] * np.asarray(W1, f32))
    b1_f = np.asarray(norm2_b, f32) @ np.asarray(W1, f32) + np.asarray(b1, f32)

    common = {
        "wq": Wq_f.astype(BF), "wk": Wk_f.astype(BF), "wv": Wv_f.astype(BF),
        "wo": np.asarray(Wo, f32).astype(BF),
        "w1": W1_f.astype(BF), "w2": np.asarray(W2, f32).astype(BF),
        "bq": np.ascontiguousarray(bq_f.reshape(2, 128).T.astype(f32)),
        "bk": np.ascontiguousarray(bk_f.reshape(2, 128).T.astype(f32)),
        "bva": _bva(bv_f),
        "bo": np.ascontiguousarray(np.asarray(bo, f32).reshape(2, 128).T),
        "b1": np.ascontiguousarray(b1_f.reshape(4, 128).T.astype(f32)),
        "b2": np.ascontiguousarray(np.asarray(b2, f32).reshape(2, 128).T),
        "idb": np.eye(128, dtype=f32).astype(BF),
    }

    in_maps = []
    for c in range(8):
        b = c // 2
        qh = c % 2
        qs = qh * NQ
        perm = np.r_[qs:qs + NQ, (0 if qh else NQ):(NQ if qh else N)]
        toks = np.ascontiguousarray(tokens[b][perm])
        impp = np.ascontiguousarray(importance[b][perm].reshape(KC, 128).T.astype(f32))
        in_maps.append({"tokens": toks, "imp": impp, **common})
    return in_maps


def _run(in_maps, trace=False):
    nc = _get_compiled()
    return run_bass_kernel_spmd(nc, in_maps, core_ids=list(range(8)), trace=trace)


def _assemble(res):
    out = np.empty((B, N, D), np.float32)
    for c in range(8):
        b = c // 2
        qs = (c % 2) * NQ
        out[b, qs:qs + NQ] = res.results[c]["out"]
    return out


def kernel(**inputs) -> np.ndarray:
    res = _run(_prep_in_maps(**inputs), trace=False)
    return _assemble(res)


def kernel_traced(**inputs):
    """Like kernel() but with NTFF profiling; returns (out, exec_time_ns)."""
    res = _run(_prep_in_maps(**inputs), trace=True)
    return _assemble(res), res.exec_time_ns
